# revision 1
# baseline (speedup 1.0000x reference)
"""Bahdanau additive attention kernel for Trainium2 (8 NeuronCores, SPMD).

Problem (hardcoded): B=32, Tq=4, S=2048, H=1024, 2H=2048, fp32 inputs.
  q  = query[:, -1, :]                      [B, H]
  k  = transpose(keys, (1, 0, 2))           [B, S, 2H]
  wq = q @ Wa_w.T + Wa_b                    [B, H]
  uk = k @ Ua_w.T + Ua_b                    [B, S, H]
  sc = tanh(wq[:, None, :] + uk) @ Va_w.T   [B, S]   (+ Va_b, which softmax cancels)
  w  = softmax(sc, axis=-1)                 [B, S]
  ctx = w @ k                               [B, 2H]
  returns (ctx [B,1,2H], w [B,1,S])

Sharding: data-parallel over batch. 8 cores x 4 batches each; weights
replicated; no cross-core communication.

Host-side prep is layout/dtype only (slice, transpose, cast to bf16, and
pre-swizzle into the exact SBUF tile layouts the kernel consumes); every
FLOP of the reference computation runs on device.

Per-core dataflow (all matmuls bf16 with fp32 PSUM accumulation):
  - keys are fed twice, pre-swizzled on host: kt (transposed, d on
    partitions) feeds the big uk matmul; kn (natural, s on partitions)
    feeds the context matmul.  One 2 MiB DMA per (batch, chunk) each.
  - uk tiles [h=128, s=512] accumulate in PSUM over 16 d-strips; ScalarE
    applies tanh(. + bias[h]) where bias = wq[b] + Wa_b + Ua_b.
  - scores via PE with Va columns as the 1-wide stationary operand; exp on
    ScalarE with free-dim accumulate for the softmax denominator.
  - per chunk, the score row is PE-transposed out of exp_row into columns
    (deferred by one chunk so PE never waits on Scalar/Vector), and the
    context accumulates in 4 dedicated PSUM banks across all chunks of a
    batch (weights normalized at the end).
"""

import numpy as np

B, TQ, S, H = 32, 4, 2048, 1024
D2 = 2 * H
NCORES = 8
BPC = B // NCORES  # batches per core

_CACHE = {}


def _build(s=S, h=H, d2=D2, bpc=BPC, schunk=512):
    """Build the per-core Bass module. Parameterized so a scaled-down config
    can run in CoreSim; the shipped kernel uses the defaults."""
    from contextlib import ExitStack

    import concourse.bacc as bacc
    import concourse.mybir as mybir
    import concourse.tile as tile
    from concourse.masks import make_identity

    fp32 = mybir.dt.float32
    bf16 = mybir.dt.bfloat16
    AF = mybir.ActivationFunctionType
    SD = d2 // 128        # contraction strips for uk (d on partitions)
    SM = h // 128         # h tiles (uk output partitions / Va strips)
    SJ = h // 128         # contraction strips for wq
    NCH = s // schunk     # score chunks per batch
    SPC = schunk // 128   # keys strips per chunk
    NDC = max(1, d2 // 512)   # context output chunks
    DW = min(512, d2)         # context output chunk width
    NWH = max(1, h // 512)    # wq output chunks
    WW = min(512, h)          # wq output chunk width
    NST = s // 128            # keys strips per batch

    nc = bacc.Bacc(
        "TRN2", target_bir_lowering=False, enable_partition_id=False
    )

    qt_in = nc.dram_tensor("qt", [128, SJ, bpc], bf16, kind="ExternalInput").ap()
    kn_in = nc.dram_tensor(
        "kn", [bpc * NCH, 128, SPC, d2], bf16, kind="ExternalInput"
    ).ap()
    kt_in = nc.dram_tensor(
        "kt", [bpc * NCH, 128, SD, schunk], bf16, kind="ExternalInput"
    ).ap()
    uat_in = nc.dram_tensor("uat", [128, SD, h], bf16, kind="ExternalInput").ap()
    wat_in = nc.dram_tensor("wat", [128, SJ, h], bf16, kind="ExternalInput").ap()
    vac_in = nc.dram_tensor("vac", [128, SM], bf16, kind="ExternalInput").ap()
    wabc_in = nc.dram_tensor("wabc", [128, SM], fp32, kind="ExternalInput").ap()
    uabc_in = nc.dram_tensor("uabc", [128, SM], fp32, kind="ExternalInput").ap()
    ctx_out = nc.dram_tensor("ctx", [bpc, d2], fp32, kind="ExternalOutput").ap()
    w_out = nc.dram_tensor("wts", [bpc, s], fp32, kind="ExternalOutput").ap()

    with tile.TileContext(nc) as tc:
        with ExitStack() as ctx:
            consts = ctx.enter_context(tc.tile_pool(name="consts", bufs=1))
            knp = ctx.enter_context(tc.tile_pool(name="knp", bufs=3))
            ktp = ctx.enter_context(tc.tile_pool(name="ktp", bufs=2))
            tp = ctx.enter_context(tc.tile_pool(name="tp", bufs=SM + 1))
            rows = ctx.enter_context(tc.tile_pool(name="rows", bufs=2))
            rows2 = ctx.enter_context(tc.tile_pool(name="rows2", bufs=2))
            ps_uk = ctx.enter_context(tc.tile_pool(name="ps_uk", bufs=3, space="PSUM"))
            ps_sc = ctx.enter_context(tc.tile_pool(name="ps_sc", bufs=2, space="PSUM"))
            ps_cx = ctx.enter_context(
                tc.tile_pool(name="ps_cx", bufs=3, space="PSUM")
            )

            # ---------------- one-time setup ----------------
            ident = consts.tile([128, 128], fp32)
            make_identity(nc, ident)

            # small vectors first (gpsimd queue): qt/wat gate the wq chain
            qt = consts.tile([128, SJ, bpc], bf16)
            nc.gpsimd.dma_start(out=qt, in_=qt_in)
            # Wa^T (gpsimd; only gates the wq chain)
            wat = consts.tile([128, SJ, h], bf16)
            nc.gpsimd.dma_start(out=wat, in_=wat_in)
            vac = consts.tile([128, SM], bf16)
            nc.gpsimd.dma_start(out=vac, in_=vac_in)
            wabc = consts.tile([128, SM], fp32)
            nc.gpsimd.dma_start(out=wabc, in_=wabc_in)
            uabc = consts.tile([128, SM], fp32)
            nc.gpsimd.dma_start(out=uabc, in_=uabc_in)

            seq = [(b, c) for b in range(bpc) for c in range(NCH)]

            ktg_tiles = {}
            kn_tiles = {}

            def load_ktg(pos):
                b, c = seq[pos]
                t = ktp.tile(
                    [128, SD, schunk], bf16, tag="ktg", name=f"ktg_{b}_{c}"
                )
                nc.sync.dma_start(out=t, in_=kt_in[b * NCH + c])
                ktg_tiles[pos] = t

            def load_kn(pos, queue):
                b, c = seq[pos]
                t = knp.tile([128, SPC, d2], bf16, tag="kn", name=f"kn_{b}_{c}")
                queue.dma_start(out=t, in_=kn_in[b * NCH + c])
                kn_tiles[pos] = t

            # Startup-critical loads on sync, interleaved at d-strip-group
            # granularity so chunk 0's d-outer matmuls can trickle behind the
            # DMA front: [uat d0-1, d2-3, ktg0 d0-3, uat d4-5, ...].
            uat = consts.tile([128, SD, h], bf16)
            ktg0 = ktp.tile([128, SD, schunk], bf16, tag="ktg", name="ktg_0_0")
            ktg_tiles[0] = ktg0
            sd2 = max(1, SD // 8)
            sd4 = max(1, SD // 4)
            ug = [(g, min(g + sd2, SD)) for g in range(0, SD, sd2)]
            kq = [(g, min(g + sd4, SD)) for g in range(0, SD, sd4)]
            while ug or kq:
                if ug:
                    a, b_ = ug.pop(0)
                    nc.sync.dma_start(
                        out=uat[:, a:b_, :], in_=uat_in[:, a:b_, :]
                    )
                if kq:
                    a, b_ = kq.pop(0)
                    nc.sync.dma_start(
                        out=ktg0[:, a:b_, :], in_=kt_in[0][:, a:b_, :]
                    )
                if ug:
                    a, b_ = ug.pop(0)
                    nc.sync.dma_start(
                        out=uat[:, a:b_, :], in_=uat_in[:, a:b_, :]
                    )
            if len(seq) > 1:
                load_ktg(1)
            # First kn chunks go on sync BEHIND the critical path (they are
            # not needed until the deferred context of chunk 0/1/2), so they
            # don't steal HBM bandwidth from uat/ktg0.
            for p in range(min(3, len(seq))):
                load_kn(p, nc.sync)

            # combined additive bias columns (Wa_b + Ua_b)
            comb = consts.tile([128, SM], fp32)
            nc.vector.tensor_tensor(
                out=comb, in0=wabc, in1=uabc, op=mybir.AluOpType.add
            )

            # wq = q @ Wa^T, computed as [bpc, h] with q^T strips stationary
            wq_sb = rows.tile([bpc, h], fp32, tag="wq")
            for wh in range(NWH):
                pw = ps_uk.tile([bpc, WW], fp32, tag="uk")
                for jj in range(SJ):
                    nc.tensor.matmul(
                        out=pw,
                        lhsT=qt[:, jj, :],
                        rhs=wat[:, jj, wh * WW : (wh + 1) * WW],
                        start=(jj == 0),
                        stop=(jj == SJ - 1),
                    )
                nc.vector.tensor_copy(out=wq_sb[:, wh * WW : (wh + 1) * WW], in_=pw)

            # bias_cols[:, m, b] = wq[b, 128m:128m+128].T + (Wa_b + Ua_b) cols
            bias_cols = consts.tile([128, SM, bpc], fp32)
            for m in range(SM):
                pt = ps_sc.tile([128, bpc], fp32, tag="sc")
                nc.tensor.transpose(
                    out=pt,
                    in_=wq_sb[:bpc, m * 128 : (m + 1) * 128],
                    identity=ident[:bpc, :bpc],
                )
                nc.vector.tensor_scalar_add(
                    out=bias_cols[:, m, :], in0=pt, scalar1=comb[:, m : m + 1]
                )

            # ---------------- main loop over (batch, chunk) ----------------
            state = {}

            def new_batch_state(b):
                state[b] = {
                    "exp_row": rows.tile(
                        [1, s], fp32, tag="exp_row", name=f"exp_row_{b}"
                    ),
                    "tparts": rows2.tile(
                        [1, NCH], fp32, tag="tparts", name=f"tparts_{b}"
                    ),
                    "ecols": rows2.tile(
                        [128, NST], bf16, tag="ecols", name=f"ecols_{b}"
                    ),
                    "cx": None,
                }

            def emit_transposes(pos):
                # transpose chunk c's exp slice into columns (the bf16 copy
                # lands while the current chunk's uk stream is still running)
                b, c = seq[pos]
                st = state[b]
                pscT = ps_sc.tile([128, SPC], fp32, tag="sc", name=f"pscT_{pos}")
                for g in range(SPC):
                    nc.tensor.transpose(
                        out=pscT[:, g : g + 1],
                        in_=st["exp_row"][
                            :1, c * schunk + g * 128 : c * schunk + (g + 1) * 128
                        ],
                        identity=ident[:1, :1],
                    )
                nc.vector.tensor_copy(
                    out=st["ecols"][:, c * SPC : (c + 1) * SPC], in_=pscT
                )

            def emit_finish(pos):
                # accumulate chunk c's context partials into ONE PSUM bank:
                # the NDC output chunks go to column groups 0/32/64/96 via
                # tile_position, so consecutive jd matmuls run concurrently
                # on disjoint 32-column strips of the PE array
                b, c = seq[pos]
                st = state[b]
                if c == 0:
                    st["cx"] = ps_cx.tile([128, DW], fp32, tag="cx", name=f"cx_{b}")
                for i in range(SPC):
                    for jd in range(NDC):
                        nc.tensor.matmul(
                            out=st["cx"][32 * jd : 32 * jd + 1, :],
                            lhsT=st["ecols"][:, c * SPC + i : c * SPC + i + 1],
                            rhs=kn_tiles[pos][:, i, jd * DW : (jd + 1) * DW],
                            start=(c == 0 and i == 0),
                            stop=(c == NCH - 1 and i == SPC - 1),
                            tile_position=(0, 32 * jd),
                            skip_group_check=True,
                        )
                if c == NCH - 1:
                    # scale finished rows out, split across DVE and ACT so
                    # the tail chain isn't serialized on one engine
                    for jd in range(NDC):
                        if jd % 2 == 0:
                            nc.vector.tensor_scalar_mul(
                                out=st["ctx_row"][:, jd * DW : (jd + 1) * DW],
                                in0=st["cx"][32 * jd : 32 * jd + 1, :],
                                scalar1=st["invt"],
                            )
                        else:
                            nc.scalar.activation(
                                out=st["ctx_row"][:, jd * DW : (jd + 1) * DW],
                                in_=st["cx"][32 * jd : 32 * jd + 1, :],
                                func=AF.Copy,
                                scale=st["invt"],
                            )
                    q = nc.sync if b == bpc - 1 else nc.gpsimd
                    q.dma_start(out=ctx_out[b : b + 1, :], in_=st["ctx_row"])

            for pos, (b, c) in enumerate(seq):
                if c == 0:
                    new_batch_state(b)
                # prefetch (ktp bufs=2 -> one ahead; knp bufs=3 -> two ahead)
                if pos + 2 < len(seq) and (pos + 2) not in ktg_tiles:
                    load_ktg(pos + 2)
                if pos + 3 < len(seq) and (pos + 3) not in kn_tiles:
                    load_kn(pos + 3, nc.gpsimd)

                # uk tiles + tanh.  Chunk 0 runs d-outer with all 8 m-psums
                # live at once (borrowing every PSUM bank) so the PE can
                # consume uat/ktg0 d-strips as the startup DMAs land instead
                # of stalling for the full Ua^T before finishing any m-tile.
                ts_list = []
                if pos == 0:
                    pmap = [
                        (ps_uk, "uk"), (ps_uk, "uk"), (ps_uk, "uk"),
                        (ps_sc, "sc"), (ps_sc, "sc"),
                        (ps_cx, "cx"), (ps_cx, "cx"), (ps_cx, "cx"),
                    ]
                    pps = []
                    for m in range(SM):
                        pool, tag = pmap[m * 8 // SM]
                        pps.append(
                            pool.tile(
                                [128, schunk], fp32, tag=tag, name=f"puk0_{m}"
                            )
                        )
                    for dd in range(SD):
                        for m in range(SM):
                            nc.tensor.matmul(
                                out=pps[m],
                                lhsT=uat[:, dd, m * 128 : (m + 1) * 128],
                                rhs=ktg_tiles[pos][:, dd, :],
                                start=(dd == 0),
                                stop=(dd == SD - 1),
                                skip_group_check=True,
                            )
                    for m in range(SM):
                        t_sb = tp.tile(
                            [128, schunk], bf16, tag="t", name=f"t_{pos}_{m}"
                        )
                        nc.scalar.activation(
                            out=t_sb,
                            in_=pps[m],
                            func=AF.Tanh,
                            bias=bias_cols[:, m, b : b + 1],
                            scale=1.0,
                        )
                        ts_list.append(t_sb)
                else:
                    for m in range(SM):
                        puk = ps_uk.tile([128, schunk], fp32, tag="uk")
                        for dd in range(SD):
                            nc.tensor.matmul(
                                out=puk,
                                lhsT=uat[:, dd, m * 128 : (m + 1) * 128],
                                rhs=ktg_tiles[pos][:, dd, :],
                                start=(dd == 0),
                                stop=(dd == SD - 1),
                            )
                        t_sb = tp.tile(
                            [128, schunk], bf16, tag="t", name=f"t_{pos}_{m}"
                        )
                        nc.scalar.activation(
                            out=t_sb,
                            in_=puk,
                            func=AF.Tanh,
                            bias=bias_cols[:, m, b : b + 1],
                            scale=1.0,
                        )
                        ts_list.append(t_sb)
                        if m == 0 and pos > 0:
                            # hoist the previous chunk's score transposes
                            # here so their bf16 column cast (DVE) completes
                            # during this chunk's uk stream and the context
                            # matmuls below never wait on it
                            emit_transposes(pos - 1)

                def emit_scores(split):
                    # scores for this chunk.  split=True spreads the 8-strip
                    # contraction over 4 PE column groups (concurrent
                    # matmuls, partials at partitions 0/32/64/96) summed on
                    # DVE; the final chunk uses split=False so exp can read
                    # PSUM directly with no DVE chain on the tail
                    G = min(4, SM) if split else 1
                    gm = SM // G
                    psc = ps_sc.tile(
                        [128, schunk], fp32, tag="sc", name=f"psc_{pos}"
                    )
                    for r in range(gm):
                        for g in range(G):
                            m = g * gm + r
                            nc.tensor.matmul(
                                out=psc[32 * g : 32 * g + 1, :],
                                lhsT=vac[:, m : m + 1],
                                rhs=ts_list[m],
                                start=(r == 0),
                                stop=(r == gm - 1),
                                tile_position=(0, 32 * g),
                                skip_group_check=True,
                            )
                    if G > 1:
                        scs = rows2.tile(
                            [1, schunk], fp32, tag="scs", name=f"scs_{pos}"
                        )
                        nc.vector.tensor_copy(out=scs, in_=psc[0:1, :])
                        for g in range(1, G):
                            nc.vector.tensor_tensor(
                                out=scs,
                                in0=scs,
                                in1=psc[32 * g : 32 * g + 1, :],
                                op=mybir.AluOpType.add,
                            )
                    else:
                        scs = psc[0:1, :]
                    # exp row chunk (no max subtraction; scores are O(1))
                    # and the chunk's softmax partial sum
                    st = state[b]
                    nc.scalar.activation(
                        out=st["exp_row"][:, c * schunk : (c + 1) * schunk],
                        in_=scs,
                        func=AF.Exp,
                        accum_out=st["tparts"][:, c : c + 1],
                    )

                last = pos == len(seq) - 1
                if last:
                    emit_scores(split=False)
                if pos > 0:
                    emit_finish(pos - 1)
                if not last:
                    emit_scores(split=True)
                st = state[b]
                if c == NCH - 1:
                    # softmax denominator + normalized weights can go out now;
                    # the raw exp_row stays untouched for the deferred context
                    tsum = rows2.tile([1, 1], fp32, tag="tsum", name=f"tsum_{b}")
                    nc.vector.reduce_sum(
                        out=tsum, in_=st["tparts"], axis=mybir.AxisListType.X
                    )
                    invt = rows2.tile([1, 1], fp32, tag="invt", name=f"invt_{b}")
                    nc.vector.reciprocal(out=invt, in_=tsum)
                    st["invt"] = invt
                    w_norm = rows.tile([1, s], fp32, tag="w_norm", name=f"wn_{b}")
                    nc.vector.tensor_scalar_mul(
                        out=w_norm, in0=st["exp_row"], scalar1=invt
                    )
                    wq_ = nc.sync if b == bpc - 1 else nc.gpsimd
                    wq_.dma_start(out=w_out[b : b + 1, :], in_=w_norm)
                    st["ctx_row"] = rows.tile(
                        [1, d2], fp32, tag="ctx_row", name=f"cr_{b}"
                    )

            emit_transposes(len(seq) - 1)
            emit_finish(len(seq) - 1)

    nc.compile()
    return nc


def _prep_core_inputs(q_last, keys_bf, b0, bpc, s, h, d2, schunk):
    """Host-side layout prep for one core: slice this core's batches and
    swizzle into the exact DRAM layouts the kernel DMAs from. Layout/dtype
    only -- no arithmetic."""
    import ml_dtypes

    bf16 = ml_dtypes.bfloat16
    SD = d2 // 128
    SJ = h // 128
    NCH = s // schunk
    SPC = schunk // 128

    kn = np.empty((bpc * NCH, 128, SPC, d2), dtype=bf16)
    kt = np.empty((bpc * NCH, 128, SD, schunk), dtype=bf16)
    for b in range(bpc):
        ks = keys_bf[:, b0 + b, :]  # [s, d2] (strided view)
        # kn[b,c][p, i, x] = ks[c*schunk + i*128 + p, x]
        kn[b * NCH : (b + 1) * NCH] = ks.reshape(NCH, SPC, 128, d2).transpose(
            0, 2, 1, 3
        )
        # kt[b,c][p, dd, x] = ks[c*schunk + x, dd*128 + p]
        kt[b * NCH : (b + 1) * NCH] = ks.reshape(NCH, schunk, SD, 128).transpose(
            0, 3, 2, 1
        )

    # qt[p, j, b] = q_last[b0+b, j*128+p]
    qt = np.ascontiguousarray(
        q_last[b0 : b0 + bpc].T.reshape(SJ, 128, bpc).transpose(1, 0, 2)
    ).astype(bf16)
    return {"qt": qt, "kn": kn, "kt": kt}


def _make_in_maps(inputs):
    import ml_dtypes

    bf16 = ml_dtypes.bfloat16
    q_last = np.ascontiguousarray(
        np.asarray(inputs["query"], dtype=np.float32)[:, -1, :]
    )  # [B, H]
    keys = np.asarray(inputs["keys"], dtype=np.float32)  # [S, B, 2H]
    keys_bf = keys.astype(bf16)
    wa = np.asarray(inputs["Wa_w"], dtype=np.float32)  # [H, H]
    ua = np.asarray(inputs["Ua_w"], dtype=np.float32)  # [H, 2H]
    va = np.asarray(inputs["Va_w"], dtype=np.float32).reshape(1, H)
    wab = np.asarray(inputs["Wa_b"], dtype=np.float32).reshape(H)
    uab = np.asarray(inputs["Ua_b"], dtype=np.float32).reshape(H)

    SD = D2 // 128
    SJ = H // 128
    SM = H // 128
    # uat[p, dd, j] = Ua_w[j, dd*128+p]
    uat = np.ascontiguousarray(
        ua.T.reshape(SD, 128, H).transpose(1, 0, 2)
    ).astype(bf16)
    # wat[p, jj, ho] = Wa_w[ho, jj*128+p]
    wat = np.ascontiguousarray(
        wa.T.reshape(SJ, 128, H).transpose(1, 0, 2)
    ).astype(bf16)
    # vac[p, m] = Va_w[0, m*128+p]
    vac = np.ascontiguousarray(va.reshape(SM, 128).T).astype(bf16)
    wabc = np.ascontiguousarray(wab.reshape(SM, 128).T)
    uabc = np.ascontiguousarray(uab.reshape(SM, 128).T)

    in_maps = []
    for c in range(NCORES):
        m = _prep_core_inputs(q_last, keys_bf, c * BPC, BPC, S, H, D2, 512)
        m.update(
            {"uat": uat, "wat": wat, "vac": vac, "wabc": wabc, "uabc": uabc}
        )
        in_maps.append(m)
    return in_maps


def run(inputs, trace=False, **kwargs):
    """Run on all 8 cores; returns ((context, weights), BassKernelResults)."""
    from concourse.bass_utils import run_bass_kernel_spmd

    if "nc" not in _CACHE:
        _CACHE["nc"] = _build()
    nc = _CACHE["nc"]
    in_maps = _make_in_maps(inputs)
    res = run_bass_kernel_spmd(
        nc, in_maps, core_ids=list(range(NCORES)), trace=trace, **kwargs
    )
    context = np.empty((B, 1, D2), dtype=np.float32)
    weights = np.empty((B, 1, S), dtype=np.float32)
    for c in range(NCORES):
        b0 = c * BPC
        context[b0 : b0 + BPC, 0, :] = res.results[c]["ctx"]
        weights[b0 : b0 + BPC, 0, :] = res.results[c]["wts"]
    return (context, weights), res


def kernel(**inputs):
    out, _ = run(inputs)
    return out



# revision 2
# speedup vs baseline: 1.3940x; 1.3940x over previous
"""Bahdanau additive attention kernel for Trainium2 (8 NeuronCores, SPMD).

Problem (hardcoded): B=32, Tq=4, S=2048, H=1024, 2H=2048, fp32 inputs.
  q  = query[:, -1, :]                      [B, H]
  k  = transpose(keys, (1, 0, 2))           [B, S, 2H]
  wq = q @ Wa_w.T + Wa_b                    [B, H]
  uk = k @ Ua_w.T + Ua_b                    [B, S, H]
  sc = tanh(wq[:, None, :] + uk) @ Va_w.T   [B, S]   (+ Va_b, which softmax cancels)
  w  = softmax(sc, axis=-1)                 [B, S]
  ctx = w @ k                               [B, 2H]
  returns (ctx [B,1,2H], w [B,1,S])

Sharding: data-parallel over batch. 8 cores x 4 batches each; weights
replicated; no cross-core communication.

Host-side prep is layout/dtype only (slice, transpose, permute h, cast to
bf16/fp8-e4m3, and pre-swizzle into the exact SBUF tile layouts the kernel
consumes); every FLOP of the reference computation runs on device.

Mixed-precision uk (the dominant matmul, ~85% of FLOPs):
  The additive-attention score is sum_h Va_h * tanh(...uk_h...), so the
  sensitivity of the output to noise in uk row h scales with Va_h^2.  The h
  axis is permuted (host-side layout) so |Va| is descending; the top 2/8
  h-tiles (72% of the Va^2 energy) compute uk in bf16, the bottom 6/8 run
  fp8-e4m3 with DoubleRow perf mode (2 contraction strips per PE pass, 2x
  throughput).  fp8 operands are pre-scaled (Ua by 64) and the descale is
  folded into the tanh activation's scale argument.

Per-core dataflow (fp32 PSUM accumulation everywhere):
  - keys are fed three ways, pre-swizzled on host: ktb (transposed, d on
    partitions, bf16) feeds the top-tile uk matmuls; kt8 (same layout,
    e4m3) feeds the fp8 DoubleRow uk matmuls; kn (natural, s on
    partitions, bf16) feeds the context matmul.
  - uk tiles [h=128, s=512] accumulate in PSUM (16 bf16 strips for top
    tiles, 8 DoubleRow passes for fp8 tiles); ScalarE applies
    tanh(scale*. + bias[h]) where bias = wq[b] + Wa_b + Ua_b.
  - scores via PE with Va columns as the 1-wide stationary operand; exp on
    ScalarE with free-dim accumulate for the softmax denominator.
  - per chunk, the score row is PE-transposed out of exp_row into columns
    (deferred by one chunk so PE never waits on Scalar/Vector), and the
    context accumulates in PSUM across all chunks of a batch (weights
    normalized at the end).
"""

import numpy as np

B, TQ, S, H = 32, 4, 2048, 1024
D2 = 2 * H
NCORES = 8
BPC = B // NCORES  # batches per core
MTOP = 2           # h-tiles (of 8) computed in bf16; rest fp8 DoubleRow
FP8_SCALE = 64.0   # Ua pre-scale before e4m3 cast (descale folded into tanh)

_CACHE = {}


def _build(s=S, h=H, d2=D2, bpc=BPC, schunk=512, mtop=MTOP):
    """Build the per-core Bass module. Parameterized so a scaled-down config
    can run in CoreSim; the shipped kernel uses the defaults."""
    from contextlib import ExitStack

    import concourse.bacc as bacc
    import concourse.mybir as mybir
    import concourse.tile as tile
    from concourse.masks import make_identity

    fp32 = mybir.dt.float32
    bf16 = mybir.dt.bfloat16
    fp8 = mybir.dt.float8e4
    AF = mybir.ActivationFunctionType
    DR = mybir.MatmulPerfMode.DoubleRow
    SD = d2 // 128        # contraction strips for uk (d on partitions)
    SM = h // 128         # h tiles (uk output partitions / Va strips)
    SJ = h // 128         # contraction strips for wq
    NCH = s // schunk     # score chunks per batch
    SPC = schunk // 128   # keys strips per chunk
    NDC = max(1, d2 // 512)   # context output chunks
    DW = min(512, d2)         # context output chunk width
    NWH = max(1, h // 512)    # wq output chunks
    WW = min(512, h)          # wq output chunk width
    NST = s // 128            # keys strips per batch
    HTOP = mtop * 128         # bf16 h columns
    H8 = h - HTOP             # fp8 h columns
    inv_s8 = 1.0 / FP8_SCALE

    nc = bacc.Bacc(
        "TRN2", target_bir_lowering=False, enable_partition_id=False
    )

    qt_in = nc.dram_tensor("qt", [128, SJ, bpc], bf16, kind="ExternalInput").ap()
    kn_in = nc.dram_tensor(
        "kn", [bpc * NCH, 128, SPC, d2], bf16, kind="ExternalInput"
    ).ap()
    ktb_in = nc.dram_tensor(
        "ktb", [bpc * NCH, 128, SD, schunk], bf16, kind="ExternalInput"
    ).ap()
    kt8_in = nc.dram_tensor(
        "kt8", [bpc * NCH, 128, SD, schunk], fp8, kind="ExternalInput"
    ).ap()
    uatb_in = nc.dram_tensor("uatb", [128, SD, HTOP], bf16, kind="ExternalInput").ap()
    uat8_in = nc.dram_tensor("uat8", [128, SD, H8], fp8, kind="ExternalInput").ap()
    wat_in = nc.dram_tensor("wat", [128, SJ, h], bf16, kind="ExternalInput").ap()
    vac_in = nc.dram_tensor("vac", [128, SM], bf16, kind="ExternalInput").ap()
    wabc_in = nc.dram_tensor("wabc", [128, SM], fp32, kind="ExternalInput").ap()
    uabc_in = nc.dram_tensor("uabc", [128, SM], fp32, kind="ExternalInput").ap()
    ctx_out = nc.dram_tensor("ctx", [bpc, d2], fp32, kind="ExternalOutput").ap()
    w_out = nc.dram_tensor("wts", [bpc, s], fp32, kind="ExternalOutput").ap()

    with tile.TileContext(nc) as tc:
        with ExitStack() as ctx:
            consts = ctx.enter_context(tc.tile_pool(name="consts", bufs=1))
            knp = ctx.enter_context(tc.tile_pool(name="knp", bufs=3))
            ktbp = ctx.enter_context(tc.tile_pool(name="ktbp", bufs=2))
            kt8p = ctx.enter_context(tc.tile_pool(name="kt8p", bufs=2))
            tp = ctx.enter_context(tc.tile_pool(name="tp", bufs=SM + 1))
            rows = ctx.enter_context(tc.tile_pool(name="rows", bufs=2))
            rows2 = ctx.enter_context(tc.tile_pool(name="rows2", bufs=2))
            ps_uk = ctx.enter_context(tc.tile_pool(name="ps_uk", bufs=3, space="PSUM"))
            ps_sc = ctx.enter_context(tc.tile_pool(name="ps_sc", bufs=2, space="PSUM"))
            ps_cx = ctx.enter_context(
                tc.tile_pool(name="ps_cx", bufs=3, space="PSUM")
            )

            # ---------------- one-time setup ----------------
            ident = consts.tile([128, 128], fp32)
            make_identity(nc, ident)

            # small vectors first (gpsimd queue): qt/wat gate the wq chain
            qt = consts.tile([128, SJ, bpc], bf16)
            nc.gpsimd.dma_start(out=qt, in_=qt_in)
            # Wa^T (gpsimd; only gates the wq chain)
            wat = consts.tile([128, SJ, h], bf16)
            nc.gpsimd.dma_start(out=wat, in_=wat_in)
            vac = consts.tile([128, SM], bf16)
            nc.gpsimd.dma_start(out=vac, in_=vac_in)
            wabc = consts.tile([128, SM], fp32)
            nc.gpsimd.dma_start(out=wabc, in_=wabc_in)
            uabc = consts.tile([128, SM], fp32)
            nc.gpsimd.dma_start(out=uabc, in_=uabc_in)

            seq = [(b, c) for b in range(bpc) for c in range(NCH)]

            ktb_tiles = {}
            kt8_tiles = {}
            kn_tiles = {}

            def load_ktg(pos):
                b, c = seq[pos]
                t = ktbp.tile(
                    [128, SD, schunk], bf16, tag="ktb", name=f"ktb_{b}_{c}"
                )
                nc.sync.dma_start(out=t, in_=ktb_in[b * NCH + c])
                ktb_tiles[pos] = t
                t8 = kt8p.tile(
                    [128, SD, schunk], fp8, tag="kt8", name=f"kt8_{b}_{c}"
                )
                nc.sync.dma_start(out=t8, in_=kt8_in[b * NCH + c])
                kt8_tiles[pos] = t8

            def load_kn(pos, queue):
                b, c = seq[pos]
                t = knp.tile([128, SPC, d2], bf16, tag="kn", name=f"kn_{b}_{c}")
                queue.dma_start(out=t, in_=kn_in[b * NCH + c])
                kn_tiles[pos] = t

            # Startup-critical loads on sync, interleaved at d-strip-pair
            # granularity so chunk 0's d-outer matmuls can trickle behind the
            # DMA front.
            uatb = consts.tile([128, SD, HTOP], bf16)
            uat8 = consts.tile([128, SD, H8], fp8)
            ktb0 = ktbp.tile([128, SD, schunk], bf16, tag="ktb", name="ktb_0_0")
            kt80 = kt8p.tile([128, SD, schunk], fp8, tag="kt8", name="kt8_0_0")
            ktb_tiles[0] = ktb0
            kt8_tiles[0] = kt80
            step = 2 if SD >= 2 else 1
            for g in range(0, SD, step):
                e = min(g + step, SD)
                nc.sync.dma_start(out=uatb[:, g:e, :], in_=uatb_in[:, g:e, :])
                nc.sync.dma_start(out=uat8[:, g:e, :], in_=uat8_in[:, g:e, :])
                nc.sync.dma_start(out=ktb0[:, g:e, :], in_=ktb_in[0][:, g:e, :])
                nc.sync.dma_start(out=kt80[:, g:e, :], in_=kt8_in[0][:, g:e, :])
            if len(seq) > 1:
                load_ktg(1)
            # First kn chunks go on sync BEHIND the critical path (they are
            # not needed until the deferred context of chunk 0/1/2), so they
            # don't steal HBM bandwidth from the uk stream.
            for p in range(min(3, len(seq))):
                load_kn(p, nc.sync)

            # combined additive bias columns (Wa_b + Ua_b)
            comb = consts.tile([128, SM], fp32)
            nc.vector.tensor_tensor(
                out=comb, in0=wabc, in1=uabc, op=mybir.AluOpType.add
            )

            # wq = q @ Wa^T, computed as [bpc, h] with q^T strips stationary
            wq_sb = rows.tile([bpc, h], fp32, tag="wq")
            for wh in range(NWH):
                pw = ps_uk.tile([bpc, WW], fp32, tag="uk")
                for jj in range(SJ):
                    nc.tensor.matmul(
                        out=pw,
                        lhsT=qt[:, jj, :],
                        rhs=wat[:, jj, wh * WW : (wh + 1) * WW],
                        start=(jj == 0),
                        stop=(jj == SJ - 1),
                    )
                nc.vector.tensor_copy(out=wq_sb[:, wh * WW : (wh + 1) * WW], in_=pw)

            # bias_cols[:, m, b] = wq[b, 128m:128m+128].T + (Wa_b + Ua_b) cols
            bias_cols = consts.tile([128, SM, bpc], fp32)
            for m in range(SM):
                pt = ps_sc.tile([128, bpc], fp32, tag="sc")
                nc.tensor.transpose(
                    out=pt,
                    in_=wq_sb[:bpc, m * 128 : (m + 1) * 128],
                    identity=ident[:bpc, :bpc],
                )
                nc.vector.tensor_scalar_add(
                    out=bias_cols[:, m, :], in0=pt, scalar1=comb[:, m : m + 1]
                )

            # ---------------- main loop over (batch, chunk) ----------------
            state = {}

            def new_batch_state(b):
                state[b] = {
                    "exp_row": rows.tile(
                        [1, s], fp32, tag="exp_row", name=f"exp_row_{b}"
                    ),
                    "tparts": rows2.tile(
                        [1, NCH], fp32, tag="tparts", name=f"tparts_{b}"
                    ),
                    "ecols": rows2.tile(
                        [128, NST], bf16, tag="ecols", name=f"ecols_{b}"
                    ),
                    "cx": None,
                }

            def emit_transposes(pos):
                # transpose chunk c's exp slice into columns (the bf16 copy
                # lands while the current chunk's uk stream is still running)
                b, c = seq[pos]
                st = state[b]
                pscT = ps_sc.tile([128, SPC], fp32, tag="sc", name=f"pscT_{pos}")
                for g in range(SPC):
                    nc.tensor.transpose(
                        out=pscT[:, g : g + 1],
                        in_=st["exp_row"][
                            :1, c * schunk + g * 128 : c * schunk + (g + 1) * 128
                        ],
                        identity=ident[:1, :1],
                    )
                nc.vector.tensor_copy(
                    out=st["ecols"][:, c * SPC : (c + 1) * SPC], in_=pscT
                )

            def emit_finish(pos):
                # accumulate chunk c's context partials into ONE PSUM bank:
                # the NDC output chunks go to column groups 0/32/64/96 via
                # tile_position, so consecutive jd matmuls run concurrently
                # on disjoint 32-column strips of the PE array
                b, c = seq[pos]
                st = state[b]
                if c == 0:
                    st["cx"] = ps_cx.tile([128, DW], fp32, tag="cx", name=f"cx_{b}")
                for i in range(SPC):
                    for jd in range(NDC):
                        nc.tensor.matmul(
                            out=st["cx"][32 * jd : 32 * jd + 1, :],
                            lhsT=st["ecols"][:, c * SPC + i : c * SPC + i + 1],
                            rhs=kn_tiles[pos][:, i, jd * DW : (jd + 1) * DW],
                            start=(c == 0 and i == 0),
                            stop=(c == NCH - 1 and i == SPC - 1),
                            tile_position=(0, 32 * jd),
                            skip_group_check=True,
                        )
                if c == NCH - 1:
                    # scale finished rows out, split across DVE and ACT so
                    # the tail chain isn't serialized on one engine
                    for jd in range(NDC):
                        if jd % 2 == 0:
                            nc.vector.tensor_scalar_mul(
                                out=st["ctx_row"][:, jd * DW : (jd + 1) * DW],
                                in0=st["cx"][32 * jd : 32 * jd + 1, :],
                                scalar1=st["invt"],
                            )
                        else:
                            nc.scalar.activation(
                                out=st["ctx_row"][:, jd * DW : (jd + 1) * DW],
                                in_=st["cx"][32 * jd : 32 * jd + 1, :],
                                func=AF.Copy,
                                scale=st["invt"],
                            )
                    q = nc.sync if b == bpc - 1 else nc.gpsimd
                    q.dma_start(out=ctx_out[b : b + 1, :], in_=st["ctx_row"])

            def emit_uk_matmuls(pos, m, puk):
                # uk accumulation for h-tile m of chunk pos: bf16 strips for
                # the top (Va-heavy) tiles, fp8 DoubleRow pairs for the rest
                if m < mtop:
                    for dd in range(SD):
                        nc.tensor.matmul(
                            out=puk,
                            lhsT=uatb[:, dd, m * 128 : (m + 1) * 128],
                            rhs=ktb_tiles[pos][:, dd, :],
                            start=(dd == 0),
                            stop=(dd == SD - 1),
                        )
                else:
                    m8 = m - mtop
                    for dd in range(0, SD, 2):
                        nc.tensor.matmul(
                            out=puk,
                            lhsT=uat8[:, dd : dd + 2, m8 * 128 : (m8 + 1) * 128],
                            rhs=kt8_tiles[pos][:, dd : dd + 2, :],
                            start=(dd == 0),
                            stop=(dd == SD - 2),
                            perf_mode=DR,
                        )

            for pos, (b, c) in enumerate(seq):
                if c == 0:
                    new_batch_state(b)
                # prefetch (kt pools bufs=2 -> one ahead; knp bufs=3 -> two ahead)
                if pos + 2 < len(seq) and (pos + 2) not in ktb_tiles:
                    load_ktg(pos + 2)
                if pos + 3 < len(seq) and (pos + 3) not in kn_tiles:
                    load_kn(pos + 3, nc.gpsimd)

                # uk tiles + tanh.  Chunk 0 runs d-outer with all 8 m-psums
                # live at once (borrowing every PSUM bank) so the PE can
                # consume uat/kt0 d-strip pairs as the startup DMAs land
                # instead of stalling for the full Ua^T before finishing any
                # m-tile.
                ts_list = []
                if pos == 0:
                    pmap = [
                        (ps_uk, "uk"), (ps_uk, "uk"), (ps_uk, "uk"),
                        (ps_sc, "sc"), (ps_sc, "sc"),
                        (ps_cx, "cx"), (ps_cx, "cx"), (ps_cx, "cx"),
                    ]
                    pps = []
                    for m in range(SM):
                        pool, tag = pmap[m * 8 // SM]
                        pps.append(
                            pool.tile(
                                [128, schunk], fp32, tag=tag, name=f"puk0_{m}"
                            )
                        )
                    for dd in range(0, SD, 2):
                        for m in range(SM):
                            if m < mtop:
                                for d1 in (dd, dd + 1):
                                    if d1 >= SD:
                                        continue
                                    nc.tensor.matmul(
                                        out=pps[m],
                                        lhsT=uatb[:, d1, m * 128 : (m + 1) * 128],
                                        rhs=ktb0[:, d1, :],
                                        start=(d1 == 0),
                                        stop=(d1 == SD - 1),
                                        skip_group_check=True,
                                    )
                            else:
                                m8 = m - mtop
                                nc.tensor.matmul(
                                    out=pps[m],
                                    lhsT=uat8[
                                        :, dd : dd + 2, m8 * 128 : (m8 + 1) * 128
                                    ],
                                    rhs=kt80[:, dd : dd + 2, :],
                                    start=(dd == 0),
                                    stop=(dd == SD - 2),
                                    perf_mode=DR,
                                    skip_group_check=True,
                                )
                    for m in range(SM):
                        t_sb = tp.tile(
                            [128, schunk], bf16, tag="t", name=f"t_{pos}_{m}"
                        )
                        nc.scalar.activation(
                            out=t_sb,
                            in_=pps[m],
                            func=AF.Tanh,
                            bias=bias_cols[:, m, b : b + 1],
                            scale=1.0 if m < mtop else inv_s8,
                        )
                        ts_list.append(t_sb)
                else:
                    for m in range(SM):
                        puk = ps_uk.tile([128, schunk], fp32, tag="uk")
                        emit_uk_matmuls(pos, m, puk)
                        t_sb = tp.tile(
                            [128, schunk], bf16, tag="t", name=f"t_{pos}_{m}"
                        )
                        nc.scalar.activation(
                            out=t_sb,
                            in_=puk,
                            func=AF.Tanh,
                            bias=bias_cols[:, m, b : b + 1],
                            scale=1.0 if m < mtop else inv_s8,
                        )
                        ts_list.append(t_sb)
                        if m == 0 and pos > 0:
                            # hoist the previous chunk's score transposes
                            # here so their bf16 column cast (DVE) completes
                            # during this chunk's uk stream and the context
                            # matmuls below never wait on it
                            emit_transposes(pos - 1)

                def emit_scores(split):
                    # scores for this chunk.  split=True spreads the 8-strip
                    # contraction over 4 PE column groups (concurrent
                    # matmuls, partials at partitions 0/32/64/96) summed on
                    # DVE; the final chunk uses split=False so exp can read
                    # PSUM directly with no DVE chain on the tail
                    G = min(4, SM) if split else 1
                    gm = SM // G
                    psc = ps_sc.tile(
                        [128, schunk], fp32, tag="sc", name=f"psc_{pos}"
                    )
                    for r in range(gm):
                        for g in range(G):
                            m = g * gm + r
                            nc.tensor.matmul(
                                out=psc[32 * g : 32 * g + 1, :],
                                lhsT=vac[:, m : m + 1],
                                rhs=ts_list[m],
                                start=(r == 0),
                                stop=(r == gm - 1),
                                tile_position=(0, 32 * g),
                                skip_group_check=True,
                            )
                    if G > 1:
                        scs = rows2.tile(
                            [1, schunk], fp32, tag="scs", name=f"scs_{pos}"
                        )
                        nc.vector.tensor_copy(out=scs, in_=psc[0:1, :])
                        for g in range(1, G):
                            nc.vector.tensor_tensor(
                                out=scs,
                                in0=scs,
                                in1=psc[32 * g : 32 * g + 1, :],
                                op=mybir.AluOpType.add,
                            )
                    else:
                        scs = psc[0:1, :]
                    # exp row chunk (no max subtraction; scores are O(1))
                    # and the chunk's softmax partial sum
                    st = state[b]
                    nc.scalar.activation(
                        out=st["exp_row"][:, c * schunk : (c + 1) * schunk],
                        in_=scs,
                        func=AF.Exp,
                        accum_out=st["tparts"][:, c : c + 1],
                    )

                last = pos == len(seq) - 1
                if last:
                    emit_scores(split=False)
                if pos > 0:
                    emit_finish(pos - 1)
                if not last:
                    emit_scores(split=True)
                st = state[b]
                if c == NCH - 1:
                    # softmax denominator + normalized weights can go out now;
                    # the raw exp_row stays untouched for the deferred context
                    tsum = rows2.tile([1, 1], fp32, tag="tsum", name=f"tsum_{b}")
                    nc.vector.reduce_sum(
                        out=tsum, in_=st["tparts"], axis=mybir.AxisListType.X
                    )
                    invt = rows2.tile([1, 1], fp32, tag="invt", name=f"invt_{b}")
                    nc.vector.reciprocal(out=invt, in_=tsum)
                    st["invt"] = invt
                    w_norm = rows.tile([1, s], fp32, tag="w_norm", name=f"wn_{b}")
                    nc.vector.tensor_scalar_mul(
                        out=w_norm, in0=st["exp_row"], scalar1=invt
                    )
                    wq_ = nc.sync if b == bpc - 1 else nc.gpsimd
                    wq_.dma_start(out=w_out[b : b + 1, :], in_=w_norm)
                    st["ctx_row"] = rows.tile(
                        [1, d2], fp32, tag="ctx_row", name=f"cr_{b}"
                    )

            emit_transposes(len(seq) - 1)
            emit_finish(len(seq) - 1)

    nc.compile()
    return nc


def _prep_core_inputs(q_last, keys_bf, keys_f8, b0, bpc, s, h, d2, schunk):
    """Host-side layout prep for one core: slice this core's batches and
    swizzle into the exact DRAM layouts the kernel DMAs from. Layout/dtype
    only -- no arithmetic."""
    import ml_dtypes

    bf16 = ml_dtypes.bfloat16
    f8 = ml_dtypes.float8_e4m3
    SD = d2 // 128
    SJ = h // 128
    NCH = s // schunk
    SPC = schunk // 128

    kn = np.empty((bpc * NCH, 128, SPC, d2), dtype=bf16)
    ktb = np.empty((bpc * NCH, 128, SD, schunk), dtype=bf16)
    kt8 = np.empty((bpc * NCH, 128, SD, schunk), dtype=f8)
    for b in range(bpc):
        ks = keys_bf[:, b0 + b, :]  # [s, d2] (strided view)
        k8 = keys_f8[:, b0 + b, :]
        # kn[b,c][p, i, x] = ks[c*schunk + i*128 + p, x]
        kn[b * NCH : (b + 1) * NCH] = ks.reshape(NCH, SPC, 128, d2).transpose(
            0, 2, 1, 3
        )
        # kt[b,c][p, dd, x] = ks[c*schunk + x, dd*128 + p]
        ktb[b * NCH : (b + 1) * NCH] = ks.reshape(NCH, schunk, SD, 128).transpose(
            0, 3, 2, 1
        )
        kt8[b * NCH : (b + 1) * NCH] = k8.reshape(NCH, schunk, SD, 128).transpose(
            0, 3, 2, 1
        )

    # qt[p, j, b] = q_last[b0+b, j*128+p]
    qt = np.ascontiguousarray(
        q_last[b0 : b0 + bpc].T.reshape(SJ, 128, bpc).transpose(1, 0, 2)
    ).astype(bf16)
    return {"qt": qt, "kn": kn, "ktb": ktb, "kt8": kt8}


def _make_in_maps(inputs):
    import ml_dtypes

    bf16 = ml_dtypes.bfloat16
    f8 = ml_dtypes.float8_e4m3
    q_last = np.ascontiguousarray(
        np.asarray(inputs["query"], dtype=np.float32)[:, -1, :]
    )  # [B, H]
    keys = np.asarray(inputs["keys"], dtype=np.float32)  # [S, B, 2H]
    keys_bf = keys.astype(bf16)
    keys_f8 = np.clip(keys, -240.0, 240.0).astype(f8)
    wa = np.asarray(inputs["Wa_w"], dtype=np.float32)  # [H, H]
    ua = np.asarray(inputs["Ua_w"], dtype=np.float32)  # [H, 2H]
    va = np.asarray(inputs["Va_w"], dtype=np.float32).reshape(1, H)
    wab = np.asarray(inputs["Wa_b"], dtype=np.float32).reshape(H)
    uab = np.asarray(inputs["Ua_b"], dtype=np.float32).reshape(H)

    # permute the h axis so |Va| is descending: the top MTOP h-tiles (most of
    # the Va^2 energy, i.e. of the output sensitivity) run in bf16, the rest
    # in fp8.  Pure layout change; scores/outputs are h-order invariant.
    perm = np.argsort(-np.abs(va[0]), kind="stable")
    wa = wa[perm]
    ua = ua[perm]
    va = va[:, perm]
    wab = wab[perm]
    uab = uab[perm]

    SD = D2 // 128
    SJ = H // 128
    SM = H // 128
    HTOP = MTOP * 128
    # uat[p, dd, j] = Ua_w[j, dd*128+p], split into bf16 (top) / fp8 (rest)
    uat = np.ascontiguousarray(ua.T.reshape(SD, 128, H).transpose(1, 0, 2))
    uatb = uat[:, :, :HTOP].astype(bf16)
    uat8 = np.clip(uat[:, :, HTOP:] * FP8_SCALE, -240.0, 240.0).astype(f8)
    # wat[p, jj, ho] = Wa_w[ho, jj*128+p]
    wat = np.ascontiguousarray(
        wa.T.reshape(SJ, 128, H).transpose(1, 0, 2)
    ).astype(bf16)
    # vac[p, m] = Va_w[0, m*128+p]
    vac = np.ascontiguousarray(va.reshape(SM, 128).T).astype(bf16)
    wabc = np.ascontiguousarray(wab.reshape(SM, 128).T)
    uabc = np.ascontiguousarray(uab.reshape(SM, 128).T)

    in_maps = []
    for c in range(NCORES):
        m = _prep_core_inputs(
            q_last, keys_bf, keys_f8, c * BPC, BPC, S, H, D2, 512
        )
        m.update(
            {
                "uatb": uatb,
                "uat8": uat8,
                "wat": wat,
                "vac": vac,
                "wabc": wabc,
                "uabc": uabc,
            }
        )
        in_maps.append(m)
    return in_maps


def run(inputs, trace=False, **kwargs):
    """Run on all 8 cores; returns ((context, weights), BassKernelResults)."""
    from concourse.bass_utils import run_bass_kernel_spmd

    if "nc" not in _CACHE:
        _CACHE["nc"] = _build()
    nc = _CACHE["nc"]
    in_maps = _make_in_maps(inputs)
    res = run_bass_kernel_spmd(
        nc, in_maps, core_ids=list(range(NCORES)), trace=trace, **kwargs
    )
    context = np.empty((B, 1, D2), dtype=np.float32)
    weights = np.empty((B, 1, S), dtype=np.float32)
    for c in range(NCORES):
        b0 = c * BPC
        context[b0 : b0 + BPC, 0, :] = res.results[c]["ctx"]
        weights[b0 : b0 + BPC, 0, :] = res.results[c]["wts"]
    return (context, weights), res


def kernel(**inputs):
    out, _ = run(inputs)
    return out


# revision 5
# speedup vs baseline: 1.4072x; 1.0095x over previous
"""Bahdanau additive attention kernel for Trainium2 (8 NeuronCores, SPMD).

Problem (hardcoded): B=32, Tq=4, S=2048, H=1024, 2H=2048, fp32 inputs.
  q  = query[:, -1, :]                      [B, H]
  k  = transpose(keys, (1, 0, 2))           [B, S, 2H]
  wq = q @ Wa_w.T + Wa_b                    [B, H]
  uk = k @ Ua_w.T + Ua_b                    [B, S, H]
  sc = tanh(wq[:, None, :] + uk) @ Va_w.T   [B, S]   (+ Va_b, which softmax cancels)
  w  = softmax(sc, axis=-1)                 [B, S]
  ctx = w @ k                               [B, 2H]
  returns (ctx [B,1,2H], w [B,1,S])

Sharding: data-parallel over batch. 8 cores x 4 batches each; weights
replicated; no cross-core communication.

Host-side prep is layout/dtype only (slice, transpose, permute h, cast to
bf16/fp8-e4m3, and pre-swizzle into the exact SBUF tile layouts the kernel
consumes); every FLOP of the reference computation runs on device.

Mixed-precision uk (the dominant matmul, ~85% of FLOPs):
  The additive-attention score is sum_h Va_h * tanh(...uk_h...), so the
  sensitivity of the output to noise in uk row h scales with Va_h^2.  The h
  axis is permuted (host-side layout) so |Va| is descending; the top 2/8
  h-tiles (72% of the Va^2 energy) compute uk in bf16, the bottom 6/8 run
  fp8-e4m3 with DoubleRow perf mode (2 contraction strips per PE pass, 2x
  throughput).  fp8 operands are pre-scaled (Ua by 64) and the descale is
  folded into the tanh activation's scale argument.

Per-core dataflow (fp32 PSUM accumulation everywhere):
  - keys are fed three ways, pre-swizzled on host: ktb (transposed, d on
    partitions, bf16) feeds the top-tile uk matmuls; kt8 (same layout,
    e4m3) feeds the fp8 DoubleRow uk matmuls; kn (natural, s on
    partitions, bf16) feeds the context matmul.
  - uk tiles [h=128, s=512] accumulate in PSUM (16 bf16 strips for top
    tiles, 8 DoubleRow passes for fp8 tiles); ScalarE applies
    tanh(scale*. + bias[h]) where bias = wq[b] + Wa_b + Ua_b.
  - scores via PE with Va columns as the 1-wide stationary operand; exp on
    ScalarE with free-dim accumulate for the softmax denominator.
  - per chunk, the score row is PE-transposed out of exp_row into columns
    (deferred by one chunk so PE never waits on Scalar/Vector), and the
    context accumulates in PSUM across all chunks of a batch (weights
    normalized at the end).
"""

import numpy as np

B, TQ, S, H = 32, 4, 2048, 1024
D2 = 2 * H
NCORES = 8
BPC = B // NCORES  # batches per core
MTOP = 2           # h-tiles (of 8) computed in bf16; rest fp8 DoubleRow
FP8_SCALE = 64.0   # Ua pre-scale before e4m3 cast (descale folded into tanh)

_CACHE = {}


def _build(s=S, h=H, d2=D2, bpc=BPC, schunk=512, mtop=MTOP):
    """Build the per-core Bass module. Parameterized so a scaled-down config
    can run in CoreSim; the shipped kernel uses the defaults."""
    from contextlib import ExitStack

    import concourse.bacc as bacc
    import concourse.mybir as mybir
    import concourse.tile as tile
    from concourse.masks import make_identity

    fp32 = mybir.dt.float32
    bf16 = mybir.dt.bfloat16
    fp8 = mybir.dt.float8e4
    AF = mybir.ActivationFunctionType
    DR = mybir.MatmulPerfMode.DoubleRow
    SD = d2 // 128        # contraction strips for uk (d on partitions)
    SM = h // 128         # h tiles (uk output partitions / Va strips)
    SJ = h // 128         # contraction strips for wq
    NCH = s // schunk     # score chunks per batch
    SPC = schunk // 128   # keys strips per chunk
    NDC = max(1, d2 // 512)   # context output chunks
    DW = min(512, d2)         # context output chunk width
    NWH = max(1, h // 512)    # wq output chunks
    WW = min(512, h)          # wq output chunk width
    NST = s // 128            # keys strips per batch
    HTOP = mtop * 128         # bf16 h columns
    H8 = h - HTOP             # fp8 h columns
    inv_s8 = 1.0 / FP8_SCALE

    nc = bacc.Bacc(
        "TRN2", target_bir_lowering=False, enable_partition_id=False
    )

    qt_in = nc.dram_tensor("qt", [128, SJ, bpc], bf16, kind="ExternalInput").ap()
    kn_in = nc.dram_tensor(
        "kn", [bpc * NCH, 128, SPC, d2], bf16, kind="ExternalInput"
    ).ap()
    ktb_in = nc.dram_tensor(
        "ktb", [bpc * NCH, 128, SD, schunk], bf16, kind="ExternalInput"
    ).ap()
    kt8_in = nc.dram_tensor(
        "kt8", [bpc * NCH, 128, SD, schunk], fp8, kind="ExternalInput"
    ).ap()
    uatb_in = nc.dram_tensor("uatb", [128, SD, HTOP], bf16, kind="ExternalInput").ap()
    uat8_in = nc.dram_tensor("uat8", [128, SD, H8], fp8, kind="ExternalInput").ap()
    wat_in = nc.dram_tensor("wat", [128, SJ, h], bf16, kind="ExternalInput").ap()
    vac_in = nc.dram_tensor("vac", [128, SM], bf16, kind="ExternalInput").ap()
    wabc_in = nc.dram_tensor("wabc", [128, SM], fp32, kind="ExternalInput").ap()
    uabc_in = nc.dram_tensor("uabc", [128, SM], fp32, kind="ExternalInput").ap()
    ctx_out = nc.dram_tensor("ctx", [bpc, d2], fp32, kind="ExternalOutput").ap()
    w_out = nc.dram_tensor("wts", [bpc, s], fp32, kind="ExternalOutput").ap()

    with tile.TileContext(nc) as tc:
        with ExitStack() as ctx:
            consts = ctx.enter_context(tc.tile_pool(name="consts", bufs=1))
            knp = ctx.enter_context(tc.tile_pool(name="knp", bufs=3))
            ktbp = ctx.enter_context(tc.tile_pool(name="ktbp", bufs=2))
            kt8p = ctx.enter_context(tc.tile_pool(name="kt8p", bufs=2))
            tp = ctx.enter_context(tc.tile_pool(name="tp", bufs=SM + 1))
            rows = ctx.enter_context(tc.tile_pool(name="rows", bufs=2))
            rows2 = ctx.enter_context(tc.tile_pool(name="rows2", bufs=2))
            ps_uk = ctx.enter_context(tc.tile_pool(name="ps_uk", bufs=3, space="PSUM"))
            ps_sc = ctx.enter_context(tc.tile_pool(name="ps_sc", bufs=2, space="PSUM"))
            ps_cx = ctx.enter_context(
                tc.tile_pool(name="ps_cx", bufs=3, space="PSUM")
            )

            # ---------------- one-time setup ----------------
            ident = consts.tile([128, 128], fp32)
            make_identity(nc, ident)

            # small vectors first (gpsimd queue): qt/wat gate the wq chain
            qt = consts.tile([128, SJ, bpc], bf16)
            nc.gpsimd.dma_start(out=qt, in_=qt_in)
            # Wa^T (gpsimd; only gates the wq chain)
            wat = consts.tile([128, SJ, h], bf16)
            nc.gpsimd.dma_start(out=wat, in_=wat_in)
            vac = consts.tile([128, SM], bf16)
            nc.gpsimd.dma_start(out=vac, in_=vac_in)
            wabc = consts.tile([128, SM], fp32)
            nc.gpsimd.dma_start(out=wabc, in_=wabc_in)
            uabc = consts.tile([128, SM], fp32)
            nc.gpsimd.dma_start(out=uabc, in_=uabc_in)

            seq = [(b, c) for b in range(bpc) for c in range(NCH)]

            ktb_tiles = {}
            kt8_tiles = {}
            kn_tiles = {}

            def load_ktg(pos):
                # fp8 first: each chunk's m-loop starts on the fp8 tiles, so
                # the smaller tensor landing first hides DMA jitter
                b, c = seq[pos]
                t8 = kt8p.tile(
                    [128, SD, schunk], fp8, tag="kt8", name=f"kt8_{b}_{c}"
                )
                nc.sync.dma_start(out=t8, in_=kt8_in[b * NCH + c])
                kt8_tiles[pos] = t8
                t = ktbp.tile(
                    [128, SD, schunk], bf16, tag="ktb", name=f"ktb_{b}_{c}"
                )
                nc.sync.dma_start(out=t, in_=ktb_in[b * NCH + c])
                ktb_tiles[pos] = t

            def load_kn(pos, queue):
                b, c = seq[pos]
                t = knp.tile([128, SPC, d2], bf16, tag="kn", name=f"kn_{b}_{c}")
                queue.dma_start(out=t, in_=kn_in[b * NCH + c])
                kn_tiles[pos] = t

            # Startup-critical loads on sync, interleaved at d-strip-pair
            # granularity so chunk 0's d-outer matmuls can trickle behind the
            # DMA front.
            uatb = consts.tile([128, SD, HTOP], bf16)
            uat8 = consts.tile([128, SD, H8], fp8)
            ktb0 = ktbp.tile([128, SD, schunk], bf16, tag="ktb", name="ktb_0_0")
            kt80 = kt8p.tile([128, SD, schunk], fp8, tag="kt8", name="kt8_0_0")
            ktb_tiles[0] = ktb0
            kt8_tiles[0] = kt80
            # fp8 halves first (chunk 0 runs its fp8 m-tiles first), then the
            # bf16 halves, so PE work starts as early as possible
            step = 2 if SD >= 2 else 1
            for g in range(0, SD, step):
                e = min(g + step, SD)
                nc.sync.dma_start(out=uat8[:, g:e, :], in_=uat8_in[:, g:e, :])
                nc.sync.dma_start(out=kt80[:, g:e, :], in_=kt8_in[0][:, g:e, :])
            for g in range(0, SD, step):
                e = min(g + step, SD)
                nc.sync.dma_start(out=uatb[:, g:e, :], in_=uatb_in[:, g:e, :])
                nc.sync.dma_start(out=ktb0[:, g:e, :], in_=ktb_in[0][:, g:e, :])
            if len(seq) > 1:
                load_ktg(1)
            # First kn chunks go on gpsimd BEHIND the critical path (they are
            # not needed until the deferred context of chunk 0/1/2), so they
            # don't delay the sync-queue ktg prefetches for positions 2-4.
            for p in range(min(3, len(seq))):
                load_kn(p, nc.gpsimd)

            # combined additive bias columns (Wa_b + Ua_b)
            comb = consts.tile([128, SM], fp32)
            nc.vector.tensor_tensor(
                out=comb, in0=wabc, in1=uabc, op=mybir.AluOpType.add
            )

            # wq = q @ Wa^T, computed as [bpc, h] with q^T strips stationary
            wq_sb = rows.tile([bpc, h], fp32, tag="wq")
            for wh in range(NWH):
                pw = ps_uk.tile([bpc, WW], fp32, tag="uk")
                for jj in range(SJ):
                    nc.tensor.matmul(
                        out=pw,
                        lhsT=qt[:, jj, :],
                        rhs=wat[:, jj, wh * WW : (wh + 1) * WW],
                        start=(jj == 0),
                        stop=(jj == SJ - 1),
                    )
                nc.vector.tensor_copy(out=wq_sb[:, wh * WW : (wh + 1) * WW], in_=pw)

            # bias_cols[:, m, b] = wq[b, 128m:128m+128].T + (Wa_b + Ua_b) cols
            bias_cols = consts.tile([128, SM, bpc], fp32)
            for m in range(SM):
                pt = ps_sc.tile([128, bpc], fp32, tag="sc")
                nc.tensor.transpose(
                    out=pt,
                    in_=wq_sb[:bpc, m * 128 : (m + 1) * 128],
                    identity=ident[:bpc, :bpc],
                )
                nc.vector.tensor_scalar_add(
                    out=bias_cols[:, m, :], in0=pt, scalar1=comb[:, m : m + 1]
                )

            # ---------------- main loop over (batch, chunk) ----------------
            state = {}

            def new_batch_state(b):
                state[b] = {
                    "exp_row": rows.tile(
                        [1, s], fp32, tag="exp_row", name=f"exp_row_{b}"
                    ),
                    "tparts": rows2.tile(
                        [1, NCH], fp32, tag="tparts", name=f"tparts_{b}"
                    ),
                    "ecols": rows2.tile(
                        [128, NST], bf16, tag="ecols", name=f"ecols_{b}"
                    ),
                    "cx": None,
                }

            def emit_transposes(pos):
                # transpose chunk c's exp slice into columns (the bf16 copy
                # lands while the current chunk's uk stream is still running)
                b, c = seq[pos]
                st = state[b]
                pscT = ps_sc.tile([128, SPC], fp32, tag="sc", name=f"pscT_{pos}")
                for g in range(SPC):
                    nc.tensor.transpose(
                        out=pscT[:, g : g + 1],
                        in_=st["exp_row"][
                            :1, c * schunk + g * 128 : c * schunk + (g + 1) * 128
                        ],
                        identity=ident[:1, :1],
                    )
                nc.vector.tensor_copy(
                    out=st["ecols"][:, c * SPC : (c + 1) * SPC], in_=pscT
                )

            def emit_finish(pos):
                # accumulate chunk c's context partials into ONE PSUM bank:
                # the NDC output chunks go to column groups 0/32/64/96 via
                # tile_position, so consecutive jd matmuls run concurrently
                # on disjoint 32-column strips of the PE array
                b, c = seq[pos]
                st = state[b]
                if c == 0:
                    st["cx"] = ps_cx.tile([128, DW], fp32, tag="cx", name=f"cx_{b}")
                for i in range(SPC):
                    for jd in range(NDC):
                        nc.tensor.matmul(
                            out=st["cx"][32 * jd : 32 * jd + 1, :],
                            lhsT=st["ecols"][:, c * SPC + i : c * SPC + i + 1],
                            rhs=kn_tiles[pos][:, i, jd * DW : (jd + 1) * DW],
                            start=(c == 0 and i == 0),
                            stop=(c == NCH - 1 and i == SPC - 1),
                            tile_position=(0, 32 * jd),
                            skip_group_check=True,
                        )
                if c == NCH - 1:
                    # scale finished rows out, split across DVE and ACT so
                    # the tail chain isn't serialized on one engine
                    for jd in range(NDC):
                        if jd % 2 == 0:
                            nc.vector.tensor_scalar_mul(
                                out=st["ctx_row"][:, jd * DW : (jd + 1) * DW],
                                in0=st["cx"][32 * jd : 32 * jd + 1, :],
                                scalar1=st["invt"],
                            )
                        else:
                            nc.scalar.activation(
                                out=st["ctx_row"][:, jd * DW : (jd + 1) * DW],
                                in_=st["cx"][32 * jd : 32 * jd + 1, :],
                                func=AF.Copy,
                                scale=st["invt"],
                            )
                    q = nc.sync if b == bpc - 1 else nc.gpsimd
                    q.dma_start(out=ctx_out[b : b + 1, :], in_=st["ctx_row"])

            def emit_uk_matmuls(pos, m, puk):
                # uk accumulation for h-tile m of chunk pos: bf16 strips for
                # the top (Va-heavy) tiles, fp8 DoubleRow pairs for the rest
                if m < mtop:
                    for dd in range(SD):
                        nc.tensor.matmul(
                            out=puk,
                            lhsT=uatb[:, dd, m * 128 : (m + 1) * 128],
                            rhs=ktb_tiles[pos][:, dd, :],
                            start=(dd == 0),
                            stop=(dd == SD - 1),
                        )
                else:
                    m8 = m - mtop
                    for dd in range(0, SD, 2):
                        nc.tensor.matmul(
                            out=puk,
                            lhsT=uat8[:, dd : dd + 2, m8 * 128 : (m8 + 1) * 128],
                            rhs=kt8_tiles[pos][:, dd : dd + 2, :],
                            start=(dd == 0),
                            stop=(dd == SD - 2),
                            perf_mode=DR,
                        )

            for pos, (b, c) in enumerate(seq):
                if c == 0:
                    new_batch_state(b)
                # prefetch (kt pools bufs=2 -> one ahead; knp bufs=3 -> two ahead)
                if pos + 2 < len(seq) and (pos + 2) not in ktb_tiles:
                    load_ktg(pos + 2)
                if pos + 3 < len(seq) and (pos + 3) not in kn_tiles:
                    load_kn(pos + 3, nc.gpsimd)

                # uk tiles + tanh.  Chunk 0 runs d-outer with all 8 m-psums
                # live at once (borrowing every PSUM bank) so the PE can
                # consume uat/kt0 d-strip pairs as the startup DMAs land
                # instead of stalling for the full Ua^T before finishing any
                # m-tile.
                # fp8 m-tiles run first (their operands land first), then the
                # bf16 top tiles
                m_order = list(range(mtop, SM)) + list(range(mtop))
                ts_list = [None] * SM
                if pos == 0:
                    pmap = [
                        (ps_uk, "uk"), (ps_uk, "uk"), (ps_uk, "uk"),
                        (ps_sc, "sc"), (ps_sc, "sc"),
                        (ps_cx, "cx"), (ps_cx, "cx"), (ps_cx, "cx"),
                    ]
                    pps = []
                    for m in range(SM):
                        pool, tag = pmap[m * 8 // SM]
                        pps.append(
                            pool.tile(
                                [128, schunk], fp32, tag=tag, name=f"puk0_{m}"
                            )
                        )
                    # phase 1: d-outer over the fp8 m-tiles as uat8/kt80 land
                    for dd in range(0, SD, 2):
                        for m in range(mtop, SM):
                            m8 = m - mtop
                            nc.tensor.matmul(
                                out=pps[m],
                                lhsT=uat8[
                                    :, dd : dd + 2, m8 * 128 : (m8 + 1) * 128
                                ],
                                rhs=kt80[:, dd : dd + 2, :],
                                start=(dd == 0),
                                stop=(dd == SD - 2),
                                perf_mode=DR,
                                skip_group_check=True,
                            )
                    # phase 2: d-outer over the bf16 top tiles as uatb/ktb0 land
                    for dd in range(SD):
                        for m in range(mtop):
                            nc.tensor.matmul(
                                out=pps[m],
                                lhsT=uatb[:, dd, m * 128 : (m + 1) * 128],
                                rhs=ktb0[:, dd, :],
                                start=(dd == 0),
                                stop=(dd == SD - 1),
                                skip_group_check=True,
                            )
                    for m in m_order:
                        t_sb = tp.tile(
                            [128, schunk], bf16, tag="t", name=f"t_{pos}_{m}"
                        )
                        nc.scalar.activation(
                            out=t_sb,
                            in_=pps[m],
                            func=AF.Tanh,
                            bias=bias_cols[:, m, b : b + 1],
                            scale=1.0 if m < mtop else inv_s8,
                        )
                        ts_list[m] = t_sb
                else:
                    for mi, m in enumerate(m_order):
                        puk = ps_uk.tile([128, schunk], fp32, tag="uk")
                        emit_uk_matmuls(pos, m, puk)
                        t_sb = tp.tile(
                            [128, schunk], bf16, tag="t", name=f"t_{pos}_{m}"
                        )
                        nc.scalar.activation(
                            out=t_sb,
                            in_=puk,
                            func=AF.Tanh,
                            bias=bias_cols[:, m, b : b + 1],
                            scale=1.0 if m < mtop else inv_s8,
                        )
                        ts_list[m] = t_sb
                        if mi == 0 and pos > 0:
                            # hoist the previous chunk's score transposes
                            # here so their bf16 column cast (DVE) completes
                            # during this chunk's uk stream and the context
                            # matmuls below never wait on it
                            emit_transposes(pos - 1)

                def emit_scores(split):
                    # scores for this chunk.  split=True spreads the 8-strip
                    # contraction over 4 PE column groups (concurrent
                    # matmuls, partials at partitions 0/32/64/96) summed on
                    # DVE; the final chunk uses split=False so exp can read
                    # PSUM directly with no DVE chain on the tail
                    G = min(4, SM) if split else 1
                    gm = SM // G
                    psc = ps_sc.tile(
                        [128, schunk], fp32, tag="sc", name=f"psc_{pos}"
                    )
                    for r in range(gm):
                        for g in range(G):
                            m = g * gm + r
                            nc.tensor.matmul(
                                out=psc[32 * g : 32 * g + 1, :],
                                lhsT=vac[:, m : m + 1],
                                rhs=ts_list[m],
                                start=(r == 0),
                                stop=(r == gm - 1),
                                tile_position=(0, 32 * g),
                                skip_group_check=True,
                            )
                    if G > 1:
                        scs = rows2.tile(
                            [1, schunk], fp32, tag="scs", name=f"scs_{pos}"
                        )
                        nc.vector.tensor_copy(out=scs, in_=psc[0:1, :])
                        for g in range(1, G):
                            nc.vector.tensor_tensor(
                                out=scs,
                                in0=scs,
                                in1=psc[32 * g : 32 * g + 1, :],
                                op=mybir.AluOpType.add,
                            )
                    else:
                        scs = psc[0:1, :]
                    # exp row chunk (no max subtraction; scores are O(1))
                    # and the chunk's softmax partial sum
                    st = state[b]
                    nc.scalar.activation(
                        out=st["exp_row"][:, c * schunk : (c + 1) * schunk],
                        in_=scs,
                        func=AF.Exp,
                        accum_out=st["tparts"][:, c : c + 1],
                    )

                last = pos == len(seq) - 1
                if last:
                    emit_scores(split=False)
                if pos > 0:
                    emit_finish(pos - 1)
                if not last:
                    emit_scores(split=True)
                st = state[b]
                if c == NCH - 1:
                    # softmax denominator + normalized weights can go out now;
                    # the raw exp_row stays untouched for the deferred context
                    tsum = rows2.tile([1, 1], fp32, tag="tsum", name=f"tsum_{b}")
                    nc.vector.reduce_sum(
                        out=tsum, in_=st["tparts"], axis=mybir.AxisListType.X
                    )
                    invt = rows2.tile([1, 1], fp32, tag="invt", name=f"invt_{b}")
                    nc.vector.reciprocal(out=invt, in_=tsum)
                    st["invt"] = invt
                    w_norm = rows.tile([1, s], fp32, tag="w_norm", name=f"wn_{b}")
                    nc.vector.tensor_scalar_mul(
                        out=w_norm, in0=st["exp_row"], scalar1=invt
                    )
                    wq_ = nc.sync if b == bpc - 1 else nc.gpsimd
                    wq_.dma_start(out=w_out[b : b + 1, :], in_=w_norm)
                    st["ctx_row"] = rows.tile(
                        [1, d2], fp32, tag="ctx_row", name=f"cr_{b}"
                    )

            emit_transposes(len(seq) - 1)
            emit_finish(len(seq) - 1)

    nc.compile()
    return nc


def _prep_core_inputs(q_last, keys_bf, keys_f8, b0, bpc, s, h, d2, schunk):
    """Host-side layout prep for one core: slice this core's batches and
    swizzle into the exact DRAM layouts the kernel DMAs from. Layout/dtype
    only -- no arithmetic."""
    import ml_dtypes

    bf16 = ml_dtypes.bfloat16
    f8 = ml_dtypes.float8_e4m3
    SD = d2 // 128
    SJ = h // 128
    NCH = s // schunk
    SPC = schunk // 128

    kn = np.empty((bpc * NCH, 128, SPC, d2), dtype=bf16)
    ktb = np.empty((bpc * NCH, 128, SD, schunk), dtype=bf16)
    kt8 = np.empty((bpc * NCH, 128, SD, schunk), dtype=f8)
    for b in range(bpc):
        ks = keys_bf[:, b0 + b, :]  # [s, d2] (strided view)
        k8 = keys_f8[:, b0 + b, :]
        # kn[b,c][p, i, x] = ks[c*schunk + i*128 + p, x]
        kn[b * NCH : (b + 1) * NCH] = ks.reshape(NCH, SPC, 128, d2).transpose(
            0, 2, 1, 3
        )
        # kt[b,c][p, dd, x] = ks[c*schunk + x, dd*128 + p]
        ktb[b * NCH : (b + 1) * NCH] = ks.reshape(NCH, schunk, SD, 128).transpose(
            0, 3, 2, 1
        )
        kt8[b * NCH : (b + 1) * NCH] = k8.reshape(NCH, schunk, SD, 128).transpose(
            0, 3, 2, 1
        )

    # qt[p, j, b] = q_last[b0+b, j*128+p]
    qt = np.ascontiguousarray(
        q_last[b0 : b0 + bpc].T.reshape(SJ, 128, bpc).transpose(1, 0, 2)
    ).astype(bf16)
    return {"qt": qt, "kn": kn, "ktb": ktb, "kt8": kt8}


def _make_in_maps(inputs):
    import ml_dtypes

    bf16 = ml_dtypes.bfloat16
    f8 = ml_dtypes.float8_e4m3
    q_last = np.ascontiguousarray(
        np.asarray(inputs["query"], dtype=np.float32)[:, -1, :]
    )  # [B, H]
    keys = np.asarray(inputs["keys"], dtype=np.float32)  # [S, B, 2H]
    keys_bf = keys.astype(bf16)
    keys_f8 = np.clip(keys, -240.0, 240.0).astype(f8)
    wa = np.asarray(inputs["Wa_w"], dtype=np.float32)  # [H, H]
    ua = np.asarray(inputs["Ua_w"], dtype=np.float32)  # [H, 2H]
    va = np.asarray(inputs["Va_w"], dtype=np.float32).reshape(1, H)
    wab = np.asarray(inputs["Wa_b"], dtype=np.float32).reshape(H)
    uab = np.asarray(inputs["Ua_b"], dtype=np.float32).reshape(H)

    # permute the h axis so |Va| is descending: the top MTOP h-tiles (most of
    # the Va^2 energy, i.e. of the output sensitivity) run in bf16, the rest
    # in fp8.  Pure layout change; scores/outputs are h-order invariant.
    perm = np.argsort(-np.abs(va[0]), kind="stable")
    wa = wa[perm]
    ua = ua[perm]
    va = va[:, perm]
    wab = wab[perm]
    uab = uab[perm]

    SD = D2 // 128
    SJ = H // 128
    SM = H // 128
    HTOP = MTOP * 128
    # uat[p, dd, j] = Ua_w[j, dd*128+p], split into bf16 (top) / fp8 (rest)
    uat = np.ascontiguousarray(ua.T.reshape(SD, 128, H).transpose(1, 0, 2))
    uatb = uat[:, :, :HTOP].astype(bf16)
    uat8 = np.clip(uat[:, :, HTOP:] * FP8_SCALE, -240.0, 240.0).astype(f8)
    # wat[p, jj, ho] = Wa_w[ho, jj*128+p]
    wat = np.ascontiguousarray(
        wa.T.reshape(SJ, 128, H).transpose(1, 0, 2)
    ).astype(bf16)
    # vac[p, m] = Va_w[0, m*128+p]
    vac = np.ascontiguousarray(va.reshape(SM, 128).T).astype(bf16)
    wabc = np.ascontiguousarray(wab.reshape(SM, 128).T)
    uabc = np.ascontiguousarray(uab.reshape(SM, 128).T)

    in_maps = []
    for c in range(NCORES):
        m = _prep_core_inputs(
            q_last, keys_bf, keys_f8, c * BPC, BPC, S, H, D2, 512
        )
        m.update(
            {
                "uatb": uatb,
                "uat8": uat8,
                "wat": wat,
                "vac": vac,
                "wabc": wabc,
                "uabc": uabc,
            }
        )
        in_maps.append(m)
    return in_maps


def run(inputs, trace=False, **kwargs):
    """Run on all 8 cores; returns ((context, weights), BassKernelResults)."""
    from concourse.bass_utils import run_bass_kernel_spmd

    if "nc" not in _CACHE:
        _CACHE["nc"] = _build()
    nc = _CACHE["nc"]
    in_maps = _make_in_maps(inputs)
    res = run_bass_kernel_spmd(
        nc, in_maps, core_ids=list(range(NCORES)), trace=trace, **kwargs
    )
    context = np.empty((B, 1, D2), dtype=np.float32)
    weights = np.empty((B, 1, S), dtype=np.float32)
    for c in range(NCORES):
        b0 = c * BPC
        context[b0 : b0 + BPC, 0, :] = res.results[c]["ctx"]
        weights[b0 : b0 + BPC, 0, :] = res.results[c]["wts"]
    return (context, weights), res


def kernel(**inputs):
    out, _ = run(inputs)
    return out


# revision 11
# speedup vs baseline: 1.4455x; 1.0272x over previous
"""Bahdanau additive attention kernel for Trainium2 (8 NeuronCores, SPMD).

Problem (hardcoded): B=32, Tq=4, S=2048, H=1024, 2H=2048, fp32 inputs.
  q  = query[:, -1, :]                      [B, H]
  k  = transpose(keys, (1, 0, 2))           [B, S, 2H]
  wq = q @ Wa_w.T + Wa_b                    [B, H]
  uk = k @ Ua_w.T + Ua_b                    [B, S, H]
  sc = tanh(wq[:, None, :] + uk) @ Va_w.T   [B, S]   (+ Va_b, which softmax cancels)
  w  = softmax(sc, axis=-1)                 [B, S]
  ctx = w @ k                               [B, 2H]
  returns (ctx [B,1,2H], w [B,1,S])

Sharding: data-parallel over batch. 8 cores x 4 batches each; weights
replicated; no cross-core communication.

Host-side prep is layout/dtype only (slice, transpose, permute h, cast to
bf16/fp8-e4m3, and pre-swizzle into the exact SBUF tile layouts the kernel
consumes); every FLOP of the reference computation runs on device.

Mixed-precision uk (the dominant matmul, ~85% of FLOPs):
  The additive-attention score is sum_h Va_h * tanh(...uk_h...), so the
  sensitivity of the output to noise in uk row h scales with Va_h^2.  The h
  axis is permuted (host-side layout) so |Va| is descending; h-tile 0 (50%
  of the Va^2 energy) computes uk in bf16, h-tile 1 (21%) runs bf16 on
  NBF1/16 of the contraction strips and fp8 on the rest, and tiles 2-7 run
  fp8-e4m3 with DoubleRow perf mode (2 contraction strips per PE pass, 2x
  throughput).  fp8 operands are pre-scaled (Ua by 64) and the descale is
  folded into the tanh activation's scale argument.

Per-core dataflow (fp32 PSUM accumulation everywhere):
  - keys are fed three ways, pre-swizzled on host: ktb (transposed, d on
    partitions, bf16) feeds the bf16 uk matmuls; kt8 (same layout, e4m3)
    feeds the fp8 DoubleRow uk matmuls; kn (natural, s on partitions,
    bf16) feeds the context matmul.
  - uk tiles [h=128, s=512] accumulate in PSUM; ScalarE applies
    tanh(scale*. + bias[h]) where bias = wq[b] + Wa_b + Ua_b.
  - chunk 0 runs d-outer with all 8 m-psums live so the PE consumes
    uat/ktg0 strips as the startup DMAs land; the wq matmuls + bias
    transposes run between the fp8 and bf16 phases (by then the gpsimd
    wat load has finished) and borrow pps[0]'s PSUM bank before m0's
    matmuls overwrite it.
  - scores via PE with Va columns as the 1-wide stationary operand; exp on
    ScalarE with free-dim accumulate for the softmax denominator.
  - per chunk, the score row is PE-transposed out of exp_row into columns
    (deferred by one chunk so PE never waits on Scalar/Vector), and the
    context accumulates in PSUM across all chunks of a batch (weights
    normalized at the end; the context normalize is split into even/odd
    tiles so DVE and ScalarE run concurrently on the tail).
"""

import numpy as np

B, TQ, S, H = 32, 4, 2048, 1024
D2 = 2 * H
NCORES = 8
BPC = B // NCORES  # batches per core
NBF1 = 4           # bf16 contraction strips (of SD) on h-tile 1
FP8_SCALE = 64.0   # Ua pre-scale before e4m3 cast (descale folded into tanh)

_CACHE = {}


def _build(s=S, h=H, d2=D2, bpc=BPC, schunk=512, nbf1=NBF1):
    """Build the per-core Bass module. Parameterized so a scaled-down config
    can run in CoreSim; the shipped kernel uses the defaults."""
    from contextlib import ExitStack

    import concourse.bacc as bacc
    import concourse.mybir as mybir
    import concourse.tile as tile
    from concourse.masks import make_identity

    fp32 = mybir.dt.float32
    bf16 = mybir.dt.bfloat16
    fp8 = mybir.dt.float8e4
    AF = mybir.ActivationFunctionType
    DR = mybir.MatmulPerfMode.DoubleRow
    SD = d2 // 128        # contraction strips for uk (d on partitions)
    SM = h // 128         # h tiles (uk output partitions / Va strips)
    SJ = h // 128         # contraction strips for wq
    NCH = s // schunk     # score chunks per batch
    SPC = schunk // 128   # keys strips per chunk
    NDC = max(1, d2 // 512)   # context output chunks
    DW = min(512, d2)         # context output chunk width
    NWH = max(1, h // 512)    # wq output chunks
    WW = min(512, h)          # wq output chunk width
    NST = s // 128            # keys strips per batch
    HTOPB = min(2, SM) * 128  # uatb columns (tiles 0 and 1)
    H8 = h - 128              # uat8 columns (tiles 1..SM-1)
    inv_s8 = 1.0 / FP8_SCALE
    assert nbf1 % 2 == 0 and 0 < nbf1 < SD

    nc = bacc.Bacc(
        "TRN2", target_bir_lowering=False, enable_partition_id=False
    )

    qt_in = nc.dram_tensor("qt", [128, SJ, bpc], bf16, kind="ExternalInput").ap()
    kn_in = nc.dram_tensor(
        "kn", [bpc * NCH, 128, SPC, d2], bf16, kind="ExternalInput"
    ).ap()
    ktb_in = nc.dram_tensor(
        "ktb", [bpc * NCH, 128, SD, schunk], bf16, kind="ExternalInput"
    ).ap()
    kt8_in = nc.dram_tensor(
        "kt8", [bpc * NCH, 128, SD, schunk], fp8, kind="ExternalInput"
    ).ap()
    uatb_in = nc.dram_tensor("uatb", [128, SD, HTOPB], bf16, kind="ExternalInput").ap()
    uat8_in = nc.dram_tensor("uat8", [128, SD, H8], fp8, kind="ExternalInput").ap()
    wat_in = nc.dram_tensor("wat", [128, SJ, h], bf16, kind="ExternalInput").ap()
    vac_in = nc.dram_tensor("vac", [128, SM], bf16, kind="ExternalInput").ap()
    wabc_in = nc.dram_tensor("wabc", [128, SM], fp32, kind="ExternalInput").ap()
    uabc_in = nc.dram_tensor("uabc", [128, SM], fp32, kind="ExternalInput").ap()
    ctx_out = nc.dram_tensor("ctx", [bpc, d2], fp32, kind="ExternalOutput").ap()
    w_out = nc.dram_tensor("wts", [bpc, s], fp32, kind="ExternalOutput").ap()

    with tile.TileContext(nc) as tc:
        with ExitStack() as ctx:
            consts = ctx.enter_context(tc.tile_pool(name="consts", bufs=1))
            knp = ctx.enter_context(tc.tile_pool(name="knp", bufs=3))
            ktbp = ctx.enter_context(tc.tile_pool(name="ktbp", bufs=2))
            kt8p = ctx.enter_context(tc.tile_pool(name="kt8p", bufs=2))
            tp = ctx.enter_context(tc.tile_pool(name="tp", bufs=SM + 1))
            rows = ctx.enter_context(tc.tile_pool(name="rows", bufs=2))
            rows2 = ctx.enter_context(tc.tile_pool(name="rows2", bufs=2))
            ps_uk = ctx.enter_context(tc.tile_pool(name="ps_uk", bufs=3, space="PSUM"))
            ps_sc = ctx.enter_context(tc.tile_pool(name="ps_sc", bufs=2, space="PSUM"))
            ps_cx = ctx.enter_context(
                tc.tile_pool(name="ps_cx", bufs=3, space="PSUM")
            )

            # ---------------- one-time setup ----------------
            ident = consts.tile([128, 128], fp32)
            make_identity(nc, ident)

            # small vectors first (gpsimd queue): qt/wat gate the wq chain,
            # which runs mid-chunk-0
            qt = consts.tile([128, SJ, bpc], bf16)
            nc.gpsimd.dma_start(out=qt, in_=qt_in)
            wat = consts.tile([128, SJ, h], bf16)
            nc.gpsimd.dma_start(out=wat, in_=wat_in)
            vac = consts.tile([128, SM], bf16)
            nc.gpsimd.dma_start(out=vac, in_=vac_in)
            wabc = consts.tile([128, SM], fp32)
            nc.gpsimd.dma_start(out=wabc, in_=wabc_in)
            uabc = consts.tile([128, SM], fp32)
            nc.gpsimd.dma_start(out=uabc, in_=uabc_in)

            seq = [(b, c) for b in range(bpc) for c in range(NCH)]

            ktb_tiles = {}
            kt8_tiles = {}
            kn_tiles = {}

            def load_ktg(pos):
                # fp8 first: each chunk's m-loop starts on the fp8 tiles, so
                # the smaller tensor landing first hides DMA jitter
                b, c = seq[pos]
                t8 = kt8p.tile(
                    [128, SD, schunk], fp8, tag="kt8", name=f"kt8_{b}_{c}"
                )
                nc.sync.dma_start(out=t8, in_=kt8_in[b * NCH + c])
                kt8_tiles[pos] = t8
                t = ktbp.tile(
                    [128, SD, schunk], bf16, tag="ktb", name=f"ktb_{b}_{c}"
                )
                nc.sync.dma_start(out=t, in_=ktb_in[b * NCH + c])
                ktb_tiles[pos] = t

            def load_kn(pos, queue):
                b, c = seq[pos]
                t = knp.tile([128, SPC, d2], bf16, tag="kn", name=f"kn_{b}_{c}")
                queue.dma_start(out=t, in_=kn_in[b * NCH + c])
                kn_tiles[pos] = t

            # Startup-critical loads on sync, interleaved at d-strip-pair
            # granularity so chunk 0's d-outer matmuls can trickle behind the
            # DMA front.  fp8 halves first (chunk 0 runs its fp8 phase
            # first), then the bf16 halves.
            uatb = consts.tile([128, SD, HTOPB], bf16)
            uat8 = consts.tile([128, SD, H8], fp8)
            ktb0 = ktbp.tile([128, SD, schunk], bf16, tag="ktb", name="ktb_0_0")
            kt80 = kt8p.tile([128, SD, schunk], fp8, tag="kt8", name="kt8_0_0")
            ktb_tiles[0] = ktb0
            kt8_tiles[0] = kt80
            step = 2 if SD >= 2 else 1
            for g in range(0, SD, step):
                e = min(g + step, SD)
                nc.sync.dma_start(out=uat8[:, g:e, :], in_=uat8_in[:, g:e, :])
                nc.sync.dma_start(out=kt80[:, g:e, :], in_=kt8_in[0][:, g:e, :])
            for g in range(0, SD, step):
                e = min(g + step, SD)
                nc.sync.dma_start(out=uatb[:, g:e, :], in_=uatb_in[:, g:e, :])
                nc.sync.dma_start(out=ktb0[:, g:e, :], in_=ktb_in[0][:, g:e, :])
            if len(seq) > 1:
                load_ktg(1)
            # First kn chunks go on gpsimd BEHIND the critical path (they are
            # not needed until the deferred context of chunk 0/1/2), so they
            # don't delay the sync-queue ktg prefetches for positions 2-4.
            for p in range(min(3, len(seq))):
                load_kn(p, nc.gpsimd)

            # combined additive bias columns (Wa_b + Ua_b)
            comb = consts.tile([128, SM], fp32)
            nc.vector.tensor_tensor(
                out=comb, in0=wabc, in1=uabc, op=mybir.AluOpType.add
            )

            # wq staging + bias columns (filled mid-chunk-0, see emit_wq_bias)
            wq_sb = rows.tile([bpc, h], fp32, tag="wq")
            bias_cols = consts.tile([128, SM, bpc], fp32)

            def emit_wq_bias(pps0):
                # wq = q @ Wa^T and bias_cols[:, m, b] = wq[b].T + Wa_b + Ua_b.
                # Runs between chunk 0's fp8 and bf16 phases; all PSUM scratch
                # borrows regions of pps0 (= pps[0]) before m0's accumulation
                # overwrites the whole bank.
                for wh in range(NWH):
                    pw = pps0[:bpc, :WW]
                    for jj in range(SJ):
                        nc.tensor.matmul(
                            out=pw,
                            lhsT=qt[:, jj, :],
                            rhs=wat[:, jj, wh * WW : (wh + 1) * WW],
                            start=(jj == 0),
                            stop=(jj == SJ - 1),
                        )
                    nc.vector.tensor_copy(
                        out=wq_sb[:, wh * WW : (wh + 1) * WW], in_=pw
                    )
                for m in range(SM):
                    pt = pps0[:, m * bpc : (m + 1) * bpc]
                    nc.tensor.transpose(
                        out=pt,
                        in_=wq_sb[:bpc, m * 128 : (m + 1) * 128],
                        identity=ident[:bpc, :bpc],
                    )
                    nc.vector.tensor_scalar_add(
                        out=bias_cols[:, m, :], in0=pt, scalar1=comb[:, m : m + 1]
                    )

            # ---------------- main loop over (batch, chunk) ----------------
            state = {}

            def new_batch_state(b):
                state[b] = {
                    "exp_row": rows.tile(
                        [1, s], fp32, tag="exp_row", name=f"exp_row_{b}"
                    ),
                    "tparts": rows2.tile(
                        [1, NCH], fp32, tag="tparts", name=f"tparts_{b}"
                    ),
                    "ecols": rows2.tile(
                        [128, NST], bf16, tag="ecols", name=f"ecols_{b}"
                    ),
                    "cx": None,
                }

            def emit_transposes(pos):
                # transpose chunk c's exp slice into columns (the bf16 copy
                # lands while the current chunk's uk stream is still running)
                b, c = seq[pos]
                st = state[b]
                pscT = ps_sc.tile([128, SPC], fp32, tag="sc", name=f"pscT_{pos}")
                for g in range(SPC):
                    nc.tensor.transpose(
                        out=pscT[:, g : g + 1],
                        in_=st["exp_row"][
                            :1, c * schunk + g * 128 : c * schunk + (g + 1) * 128
                        ],
                        identity=ident[:1, :1],
                    )
                nc.vector.tensor_copy(
                    out=st["ecols"][:, c * SPC : (c + 1) * SPC], in_=pscT
                )

            def emit_finish(pos):
                # accumulate chunk c's context partials into ONE PSUM bank:
                # the NDC output chunks go to column groups 0/32/64/96 via
                # tile_position, so consecutive jd matmuls run concurrently
                # on disjoint 32-column strips of the PE array
                b, c = seq[pos]
                st = state[b]
                if c == 0:
                    st["cx"] = ps_cx.tile([128, DW], fp32, tag="cx", name=f"cx_{b}")
                for i in range(SPC):
                    for jd in range(NDC):
                        nc.tensor.matmul(
                            out=st["cx"][32 * jd : 32 * jd + 1, :],
                            lhsT=st["ecols"][:, c * SPC + i : c * SPC + i + 1],
                            rhs=kn_tiles[pos][:, i, jd * DW : (jd + 1) * DW],
                            start=(c == 0 and i == 0),
                            stop=(c == NCH - 1 and i == SPC - 1),
                            tile_position=(0, 32 * jd),
                            skip_group_check=True,
                        )
                if c == NCH - 1:
                    # scale finished rows out; even jd chunks go to one tile
                    # via DVE, odd to another via ScalarE, so the two engines
                    # run concurrently on the tail (separate tiles avoid the
                    # same-tile write serialization)
                    for jd in range(NDC):
                        half = "ctx_ev" if jd % 2 == 0 else "ctx_od"
                        idx = jd // 2
                        dst = st[half][:, idx * DW : (idx + 1) * DW]
                        if jd % 2 == 0:
                            nc.vector.tensor_scalar_mul(
                                out=dst,
                                in0=st["cx"][32 * jd : 32 * jd + 1, :],
                                scalar1=st["invt"],
                            )
                        else:
                            nc.scalar.activation(
                                out=dst,
                                in_=st["cx"][32 * jd : 32 * jd + 1, :],
                                func=AF.Copy,
                                scale=st["invt"],
                            )
                    q = nc.sync if b == bpc - 1 else nc.gpsimd
                    if NDC == 1:
                        q.dma_start(out=ctx_out[b : b + 1, :], in_=st["ctx_ev"])
                    else:
                        # strided DRAM APs: even tile -> chunks 0,2,..; odd -> 1,3,..
                        ev_ap = ctx_out[b : b + 1, :].rearrange(
                            "1 (n two w) -> 1 two n w", two=2, w=DW
                        )
                        q.dma_start(
                            out=ev_ap[:, 0],
                            in_=st["ctx_ev"].rearrange("1 (n w) -> 1 n w", w=DW),
                        )
                        q.dma_start(
                            out=ev_ap[:, 1],
                            in_=st["ctx_od"].rearrange("1 (n w) -> 1 n w", w=DW),
                        )

            def emit_uk_matmuls(pos, m, puk):
                # uk accumulation for h-tile m: tile 0 all-bf16, tile 1 mixed
                # (fp8 DoubleRow strips first, then nbf1 bf16 strips), tiles
                # 2+ all-fp8 DoubleRow
                if m == 0:
                    for dd in range(SD):
                        nc.tensor.matmul(
                            out=puk,
                            lhsT=uatb[:, dd, :128],
                            rhs=ktb_tiles[pos][:, dd, :],
                            start=(dd == 0),
                            stop=(dd == SD - 1),
                        )
                elif m == 1:
                    for dd in range(nbf1, SD, 2):
                        nc.tensor.matmul(
                            out=puk,
                            lhsT=uat8[:, dd : dd + 2, 0:128],
                            rhs=kt8_tiles[pos][:, dd : dd + 2, :],
                            start=(dd == nbf1),
                            stop=False,
                            perf_mode=DR,
                            skip_group_check=True,
                        )
                    for dd in range(nbf1):
                        nc.tensor.matmul(
                            out=puk,
                            lhsT=uatb[:, dd, 128:256],
                            rhs=ktb_tiles[pos][:, dd, :],
                            start=False,
                            stop=(dd == nbf1 - 1),
                            skip_group_check=True,
                        )
                else:
                    m8 = m - 1
                    for dd in range(0, SD, 2):
                        nc.tensor.matmul(
                            out=puk,
                            lhsT=uat8[:, dd : dd + 2, m8 * 128 : (m8 + 1) * 128],
                            rhs=kt8_tiles[pos][:, dd : dd + 2, :],
                            start=(dd == 0),
                            stop=(dd == SD - 2),
                            perf_mode=DR,
                        )

            for pos, (b, c) in enumerate(seq):
                if c == 0:
                    new_batch_state(b)
                # prefetch (kt pools bufs=2 -> one ahead; knp bufs=3 -> two ahead)
                if pos + 2 < len(seq) and (pos + 2) not in ktb_tiles:
                    load_ktg(pos + 2)
                if pos + 3 < len(seq) and (pos + 3) not in kn_tiles:
                    load_kn(pos + 3, nc.gpsimd)

                # fp8 m-tiles run first (their operands land first), then the
                # bf16-heavy tiles
                m_order = list(range(2, SM)) + [1, 0] if SM >= 2 else [0]
                ts_list = [None] * SM
                if pos == 0:
                    # chunk 0: d-outer with all m-psums live, fp8 phase then
                    # wq/bias (needs only gpsimd-loaded qt/wat) then bf16 phase
                    pmap = [
                        (ps_uk, "uk"), (ps_uk, "uk"), (ps_uk, "uk"),
                        (ps_sc, "sc"), (ps_sc, "sc"),
                        (ps_cx, "cx"), (ps_cx, "cx"), (ps_cx, "cx"),
                    ]
                    pps = []
                    for m in range(SM):
                        pool, tag = pmap[m * 8 // SM]
                        pps.append(
                            pool.tile(
                                [128, schunk], fp32, tag=tag, name=f"puk0_{m}"
                            )
                        )
                    # phase 1: all-fp8 work (m1's upper strips + tiles 2..7)
                    for dd in range(0, SD, 2):
                        for m in range(1, SM):
                            if m == 1:
                                if dd < nbf1:
                                    continue
                                lhs = uat8[:, dd : dd + 2, 0:128]
                                st_flag = dd == nbf1
                                sp_flag = False
                            else:
                                lhs = uat8[
                                    :, dd : dd + 2, (m - 1) * 128 : m * 128
                                ]
                                st_flag = dd == 0
                                sp_flag = dd == SD - 2
                            nc.tensor.matmul(
                                out=pps[m],
                                lhsT=lhs,
                                rhs=kt80[:, dd : dd + 2, :],
                                start=st_flag,
                                stop=sp_flag,
                                perf_mode=DR,
                                skip_group_check=True,
                            )
                    # wq + bias columns, borrowing pps[0]'s bank
                    emit_wq_bias(pps[0])
                    # phase 2: bf16 work (m1's first strips, then all of m0)
                    for dd in range(nbf1):
                        nc.tensor.matmul(
                            out=pps[1],
                            lhsT=uatb[:, dd, 128:256],
                            rhs=ktb0[:, dd, :],
                            start=False,
                            stop=(dd == nbf1 - 1),
                            skip_group_check=True,
                        )
                    for dd in range(SD):
                        nc.tensor.matmul(
                            out=pps[0],
                            lhsT=uatb[:, dd, :128],
                            rhs=ktb0[:, dd, :],
                            start=(dd == 0),
                            stop=(dd == SD - 1),
                            skip_group_check=True,
                        )
                    for m in m_order:
                        t_sb = tp.tile(
                            [128, schunk], bf16, tag="t", name=f"t_{pos}_{m}"
                        )
                        nc.scalar.activation(
                            out=t_sb,
                            in_=pps[m],
                            func=AF.Tanh,
                            bias=bias_cols[:, m, b : b + 1],
                            scale=1.0 if m == 0 else inv_s8,
                        )
                        ts_list[m] = t_sb
                else:
                    for mi, m in enumerate(m_order):
                        puk = ps_uk.tile([128, schunk], fp32, tag="uk")
                        emit_uk_matmuls(pos, m, puk)
                        t_sb = tp.tile(
                            [128, schunk], bf16, tag="t", name=f"t_{pos}_{m}"
                        )
                        nc.scalar.activation(
                            out=t_sb,
                            in_=puk,
                            func=AF.Tanh,
                            bias=bias_cols[:, m, b : b + 1],
                            scale=1.0 if m == 0 else inv_s8,
                        )
                        ts_list[m] = t_sb
                        if mi == 0 and pos > 0:
                            # hoist the previous chunk's score transposes
                            # here so their bf16 column cast (DVE) completes
                            # during this chunk's uk stream and the context
                            # matmuls below never wait on it
                            emit_transposes(pos - 1)

                def emit_scores(split):
                    # scores for this chunk.  split=True spreads the 8-strip
                    # contraction over 4 PE column groups (concurrent
                    # matmuls, partials at partitions 0/32/64/96) summed on
                    # DVE; the final chunk uses split=False so exp can read
                    # PSUM directly with no DVE chain on the tail
                    G = min(4, SM) if split else 1
                    gm = SM // G
                    psc = ps_sc.tile(
                        [128, schunk], fp32, tag="sc", name=f"psc_{pos}"
                    )
                    for r in range(gm):
                        for g in range(G):
                            m = g * gm + r
                            nc.tensor.matmul(
                                out=psc[32 * g : 32 * g + 1, :],
                                lhsT=vac[:, m : m + 1],
                                rhs=ts_list[m],
                                start=(r == 0),
                                stop=(r == gm - 1),
                                tile_position=(0, 32 * g),
                                skip_group_check=True,
                            )
                    if G > 1:
                        scs = rows2.tile(
                            [1, schunk], fp32, tag="scs", name=f"scs_{pos}"
                        )
                        nc.vector.tensor_copy(out=scs, in_=psc[0:1, :])
                        for g in range(1, G):
                            nc.vector.tensor_tensor(
                                out=scs,
                                in0=scs,
                                in1=psc[32 * g : 32 * g + 1, :],
                                op=mybir.AluOpType.add,
                            )
                    else:
                        scs = psc[0:1, :]
                    # exp row chunk (no max subtraction; scores are O(1))
                    # and the chunk's softmax partial sum
                    st = state[b]
                    nc.scalar.activation(
                        out=st["exp_row"][:, c * schunk : (c + 1) * schunk],
                        in_=scs,
                        func=AF.Exp,
                        accum_out=st["tparts"][:, c : c + 1],
                    )

                last = pos == len(seq) - 1
                if last:
                    emit_scores(split=False)
                if pos > 0:
                    emit_finish(pos - 1)
                if not last:
                    emit_scores(split=True)
                st = state[b]
                if c == NCH - 1:
                    # softmax denominator + normalized weights can go out now;
                    # the raw exp_row stays untouched for the deferred context
                    tsum = rows2.tile([1, 1], fp32, tag="tsum", name=f"tsum_{b}")
                    nc.vector.reduce_sum(
                        out=tsum, in_=st["tparts"], axis=mybir.AxisListType.X
                    )
                    invt = rows2.tile([1, 1], fp32, tag="invt", name=f"invt_{b}")
                    nc.vector.reciprocal(out=invt, in_=tsum)
                    st["invt"] = invt
                    w_norm = rows.tile([1, s], fp32, tag="w_norm", name=f"wn_{b}")
                    nc.vector.tensor_scalar_mul(
                        out=w_norm, in0=st["exp_row"], scalar1=invt
                    )
                    wq_ = nc.sync if b == bpc - 1 else nc.gpsimd
                    wq_.dma_start(out=w_out[b : b + 1, :], in_=w_norm)
                    st["ctx_ev"] = rows.tile(
                        [1, ((NDC + 1) // 2) * DW], fp32, tag="ctx_ev", name=f"ce_{b}"
                    )
                    if NDC > 1:
                        st["ctx_od"] = rows.tile(
                            [1, (NDC // 2) * DW], fp32, tag="ctx_od", name=f"co_{b}"
                        )

            emit_transposes(len(seq) - 1)
            emit_finish(len(seq) - 1)

    nc.compile()
    return nc


def _prep_core_inputs(q_last, keys_bf, keys_f8, b0, bpc, s, h, d2, schunk):
    """Host-side layout prep for one core: slice this core's batches and
    swizzle into the exact DRAM layouts the kernel DMAs from. Layout/dtype
    only -- no arithmetic."""
    import ml_dtypes

    bf16 = ml_dtypes.bfloat16
    f8 = ml_dtypes.float8_e4m3
    SD = d2 // 128
    SJ = h // 128
    NCH = s // schunk
    SPC = schunk // 128

    kn = np.empty((bpc * NCH, 128, SPC, d2), dtype=bf16)
    ktb = np.empty((bpc * NCH, 128, SD, schunk), dtype=bf16)
    kt8 = np.empty((bpc * NCH, 128, SD, schunk), dtype=f8)
    for b in range(bpc):
        ks = keys_bf[:, b0 + b, :]  # [s, d2] (strided view)
        k8 = keys_f8[:, b0 + b, :]
        # kn[b,c][p, i, x] = ks[c*schunk + i*128 + p, x]
        kn[b * NCH : (b + 1) * NCH] = ks.reshape(NCH, SPC, 128, d2).transpose(
            0, 2, 1, 3
        )
        # kt[b,c][p, dd, x] = ks[c*schunk + x, dd*128 + p]
        ktb[b * NCH : (b + 1) * NCH] = ks.reshape(NCH, schunk, SD, 128).transpose(
            0, 3, 2, 1
        )
        kt8[b * NCH : (b + 1) * NCH] = k8.reshape(NCH, schunk, SD, 128).transpose(
            0, 3, 2, 1
        )

    # qt[p, j, b] = q_last[b0+b, j*128+p]
    qt = np.ascontiguousarray(
        q_last[b0 : b0 + bpc].T.reshape(SJ, 128, bpc).transpose(1, 0, 2)
    ).astype(bf16)
    return {"qt": qt, "kn": kn, "ktb": ktb, "kt8": kt8}


def _make_in_maps(inputs):
    import ml_dtypes

    bf16 = ml_dtypes.bfloat16
    f8 = ml_dtypes.float8_e4m3
    q_last = np.ascontiguousarray(
        np.asarray(inputs["query"], dtype=np.float32)[:, -1, :]
    )  # [B, H]
    keys = np.asarray(inputs["keys"], dtype=np.float32)  # [S, B, 2H]
    keys_bf = keys.astype(bf16)
    keys_f8 = np.clip(keys, -240.0, 240.0).astype(f8)
    wa = np.asarray(inputs["Wa_w"], dtype=np.float32)  # [H, H]
    ua = np.asarray(inputs["Ua_w"], dtype=np.float32)  # [H, 2H]
    va = np.asarray(inputs["Va_w"], dtype=np.float32).reshape(1, H)
    wab = np.asarray(inputs["Wa_b"], dtype=np.float32).reshape(H)
    uab = np.asarray(inputs["Ua_b"], dtype=np.float32).reshape(H)

    # permute the h axis so |Va| is descending: the top h-tiles (most of
    # the Va^2 energy, i.e. of the output sensitivity) run in bf16, the rest
    # in fp8.  Pure layout change; scores/outputs are h-order invariant.
    perm = np.argsort(-np.abs(va[0]), kind="stable")
    wa = wa[perm]
    ua = ua[perm]
    va = va[:, perm]
    wab = wab[perm]
    uab = uab[perm]

    SD = D2 // 128
    SJ = H // 128
    SM = H // 128
    HTOPB = 256
    # uat[p, dd, j] = Ua_w[j, dd*128+p]; bf16 covers tiles 0-1, fp8 tiles 1+.
    # Tile 1 mixes fp8 (x FP8_SCALE) and bf16 strips in one PSUM, so its
    # bf16 columns carry the same power-of-two pre-scale (exact in bf16);
    # the tanh descale then applies uniformly.
    uat = np.ascontiguousarray(ua.T.reshape(SD, 128, H).transpose(1, 0, 2))
    uatb = np.concatenate(
        [uat[:, :, :128], uat[:, :, 128:HTOPB] * FP8_SCALE], axis=2
    ).astype(bf16)
    uat8 = np.clip(uat[:, :, 128:] * FP8_SCALE, -240.0, 240.0).astype(f8)
    # wat[p, jj, ho] = Wa_w[ho, jj*128+p]
    wat = np.ascontiguousarray(
        wa.T.reshape(SJ, 128, H).transpose(1, 0, 2)
    ).astype(bf16)
    # vac[p, m] = Va_w[0, m*128+p]
    vac = np.ascontiguousarray(va.reshape(SM, 128).T).astype(bf16)
    wabc = np.ascontiguousarray(wab.reshape(SM, 128).T)
    uabc = np.ascontiguousarray(uab.reshape(SM, 128).T)

    in_maps = []
    for c in range(NCORES):
        m = _prep_core_inputs(
            q_last, keys_bf, keys_f8, c * BPC, BPC, S, H, D2, 512
        )
        m.update(
            {
                "uatb": uatb,
                "uat8": uat8,
                "wat": wat,
                "vac": vac,
                "wabc": wabc,
                "uabc": uabc,
            }
        )
        in_maps.append(m)
    return in_maps


def run(inputs, trace=False, **kwargs):
    """Run on all 8 cores; returns ((context, weights), BassKernelResults)."""
    from concourse.bass_utils import run_bass_kernel_spmd

    if "nc" not in _CACHE:
        _CACHE["nc"] = _build()
    nc = _CACHE["nc"]
    in_maps = _make_in_maps(inputs)
    res = run_bass_kernel_spmd(
        nc, in_maps, core_ids=list(range(NCORES)), trace=trace, **kwargs
    )
    context = np.empty((B, 1, D2), dtype=np.float32)
    weights = np.empty((B, 1, S), dtype=np.float32)
    for c in range(NCORES):
        b0 = c * BPC
        context[b0 : b0 + BPC, 0, :] = res.results[c]["ctx"]
        weights[b0 : b0 + BPC, 0, :] = res.results[c]["wts"]
    return (context, weights), res


def kernel(**inputs):
    out, _ = run(inputs)
    return out


# revision 14
# speedup vs baseline: 1.4757x; 1.0209x over previous
"""Bahdanau additive attention kernel for Trainium2 (8 NeuronCores, SPMD).

Problem (hardcoded): B=32, Tq=4, S=2048, H=1024, 2H=2048, fp32 inputs.
  q  = query[:, -1, :]                      [B, H]
  k  = transpose(keys, (1, 0, 2))           [B, S, 2H]
  wq = q @ Wa_w.T + Wa_b                    [B, H]
  uk = k @ Ua_w.T + Ua_b                    [B, S, H]
  sc = tanh(wq[:, None, :] + uk) @ Va_w.T   [B, S]   (+ Va_b, which softmax cancels)
  w  = softmax(sc, axis=-1)                 [B, S]
  ctx = w @ k                               [B, 2H]
  returns (ctx [B,1,2H], w [B,1,S])

Sharding: data-parallel over batch. 8 cores x 4 batches each; weights
replicated; no cross-core communication.

Host-side prep is layout/dtype only (slice, transpose, permute h, cast to
bf16/fp8-e4m3, and pre-swizzle into the exact SBUF tile layouts the kernel
consumes); every FLOP of the reference computation runs on device.

Mixed-precision uk (the dominant matmul, ~85% of FLOPs):
  The additive-attention score is sum_h Va_h * tanh(...uk_h...), so the
  sensitivity of the output to noise in uk row h scales with Va_h^2.  The h
  axis is permuted (host-side layout) so |Va| is descending; h-tile 0 (50%
  of the Va^2 energy) computes uk in bf16, h-tile 1 (21%) runs bf16 on
  NBF1/16 of the contraction strips and fp8 on the rest, and tiles 2-7 run
  fp8-e4m3 with DoubleRow perf mode (2 contraction strips per PE pass, 2x
  throughput).  fp8 operands are pre-scaled (Ua by 64) and the descale is
  folded into the tanh activation's scale argument.

Per-core dataflow (fp32 PSUM accumulation everywhere):
  - keys are fed three ways, pre-swizzled on host: ktb (transposed, d on
    partitions, bf16) feeds the bf16 uk matmuls; kt8 (same layout, e4m3)
    feeds the fp8 DoubleRow uk matmuls; kn (natural, s on partitions,
    bf16) feeds the context matmul.
  - uk tiles [h=128, s=512] accumulate in PSUM; ScalarE applies
    tanh(scale*. + bias[h]) where bias = wq[b] + Wa_b + Ua_b.
  - chunk 0 runs d-outer with all 8 m-psums live so the PE consumes
    uat/ktg0 strips as the startup DMAs land; the wq matmuls + bias
    transposes run between the fp8 and bf16 phases (by then the gpsimd
    wat load has finished) and borrow pps[0]'s PSUM bank before m0's
    matmuls overwrite it.
  - scores via PE with Va columns as the 1-wide stationary operand; exp on
    ScalarE with free-dim accumulate for the softmax denominator.
  - per chunk, the score row is PE-transposed out of exp_row into columns
    (deferred by one chunk so PE never waits on Scalar/Vector), and the
    context accumulates in PSUM across all chunks of a batch (weights
    normalized at the end; the context normalize is split into even/odd
    tiles so DVE and ScalarE run concurrently on the tail).
"""

import numpy as np

B, TQ, S, H = 32, 4, 2048, 1024
D2 = 2 * H
NCORES = 8
BPC = B // NCORES  # batches per core
NBF1 = 4           # bf16 contraction strips (of SD) on h-tile 1
FP8_SCALE = 64.0   # Ua pre-scale before e4m3 cast (descale folded into tanh)

_CACHE = {}


def _build(s=S, h=H, d2=D2, bpc=BPC, schunk=512, nbf1=NBF1):
    """Build the per-core Bass module. Parameterized so a scaled-down config
    can run in CoreSim; the shipped kernel uses the defaults."""
    from contextlib import ExitStack

    import concourse.bacc as bacc
    import concourse.mybir as mybir
    import concourse.tile as tile
    from concourse.masks import make_identity

    fp32 = mybir.dt.float32
    bf16 = mybir.dt.bfloat16
    fp8 = mybir.dt.float8e4
    AF = mybir.ActivationFunctionType
    DR = mybir.MatmulPerfMode.DoubleRow
    SD = d2 // 128        # contraction strips for uk (d on partitions)
    SM = h // 128         # h tiles (uk output partitions / Va strips)
    SJ = h // 128         # contraction strips for wq
    NCH = s // schunk     # score chunks per batch
    SPC = schunk // 128   # keys strips per chunk
    NDC = max(1, d2 // 512)   # context output chunks
    DW = min(512, d2)         # context output chunk width
    NWH = max(1, h // 512)    # wq output chunks
    WW = min(512, h)          # wq output chunk width
    NST = s // 128            # keys strips per batch
    HTOPB = min(2, SM) * 128  # uatb columns (tiles 0 and 1)
    H8 = h - 128              # uat8 columns (tiles 1..SM-1)
    inv_s8 = 1.0 / FP8_SCALE
    assert nbf1 % 2 == 0 and 0 < nbf1 < SD

    nc = bacc.Bacc(
        "TRN2", target_bir_lowering=False, enable_partition_id=False
    )

    qt_in = nc.dram_tensor("qt", [128, SJ, bpc], bf16, kind="ExternalInput").ap()
    kn_in = nc.dram_tensor(
        "kn", [bpc * NCH, 128, SPC, d2], bf16, kind="ExternalInput"
    ).ap()
    ktb_in = nc.dram_tensor(
        "ktb", [bpc * NCH, 128, SD, schunk], bf16, kind="ExternalInput"
    ).ap()
    kt8_in = nc.dram_tensor(
        "kt8", [bpc * NCH, 128, SD, schunk], fp8, kind="ExternalInput"
    ).ap()
    uatb_in = nc.dram_tensor("uatb", [128, SD, HTOPB], bf16, kind="ExternalInput").ap()
    uat8_in = nc.dram_tensor("uat8", [128, SD, H8], fp8, kind="ExternalInput").ap()
    wat_in = nc.dram_tensor("wat", [128, SJ, h], bf16, kind="ExternalInput").ap()
    vac_in = nc.dram_tensor("vac", [128, SM], bf16, kind="ExternalInput").ap()
    wabc_in = nc.dram_tensor("wabc", [128, SM], fp32, kind="ExternalInput").ap()
    uabc_in = nc.dram_tensor("uabc", [128, SM], fp32, kind="ExternalInput").ap()
    ctx_out = nc.dram_tensor("ctx", [bpc, d2], fp32, kind="ExternalOutput").ap()
    w_out = nc.dram_tensor("wts", [bpc, s], fp32, kind="ExternalOutput").ap()

    with tile.TileContext(nc) as tc:
        with ExitStack() as ctx:
            consts = ctx.enter_context(tc.tile_pool(name="consts", bufs=1))
            knp = ctx.enter_context(tc.tile_pool(name="knp", bufs=3))
            ktbp = ctx.enter_context(tc.tile_pool(name="ktbp", bufs=2))
            kt8p = ctx.enter_context(tc.tile_pool(name="kt8p", bufs=2))
            tp = ctx.enter_context(tc.tile_pool(name="tp", bufs=SM + 1))
            rows = ctx.enter_context(tc.tile_pool(name="rows", bufs=2))
            rows2 = ctx.enter_context(tc.tile_pool(name="rows2", bufs=2))
            ps_uk = ctx.enter_context(tc.tile_pool(name="ps_uk", bufs=3, space="PSUM"))
            ps_sc = ctx.enter_context(tc.tile_pool(name="ps_sc", bufs=2, space="PSUM"))
            ps_cx = ctx.enter_context(
                tc.tile_pool(name="ps_cx", bufs=3, space="PSUM")
            )

            # ---------------- one-time setup ----------------
            ident = consts.tile([128, 128], fp32)
            make_identity(nc, ident)

            # small vectors first (gpsimd queue): qt/wat gate the wq chain,
            # which runs mid-chunk-0
            qt = consts.tile([128, SJ, bpc], bf16)
            nc.gpsimd.dma_start(out=qt, in_=qt_in)
            wat = consts.tile([128, SJ, h], bf16)
            nc.gpsimd.dma_start(out=wat, in_=wat_in)
            vac = consts.tile([128, SM], bf16)
            nc.gpsimd.dma_start(out=vac, in_=vac_in)
            wabc = consts.tile([128, SM], fp32)
            nc.gpsimd.dma_start(out=wabc, in_=wabc_in)
            uabc = consts.tile([128, SM], fp32)
            nc.gpsimd.dma_start(out=uabc, in_=uabc_in)

            seq = [(b, c) for b in range(bpc) for c in range(NCH)]

            ktb_tiles = {}
            kt8_tiles = {}
            kn_tiles = {}

            def load_ktg(pos):
                # fp8 first: each chunk's m-loop starts on the fp8 tiles, so
                # the smaller tensor landing first hides DMA jitter
                b, c = seq[pos]
                t8 = kt8p.tile(
                    [128, SD, schunk], fp8, tag="kt8", name=f"kt8_{b}_{c}"
                )
                nc.sync.dma_start(out=t8, in_=kt8_in[b * NCH + c])
                kt8_tiles[pos] = t8
                t = ktbp.tile(
                    [128, SD, schunk], bf16, tag="ktb", name=f"ktb_{b}_{c}"
                )
                nc.sync.dma_start(out=t, in_=ktb_in[b * NCH + c])
                ktb_tiles[pos] = t

            def load_kn(pos, queue):
                b, c = seq[pos]
                t = knp.tile([128, SPC, d2], bf16, tag="kn", name=f"kn_{b}_{c}")
                queue.dma_start(out=t, in_=kn_in[b * NCH + c])
                kn_tiles[pos] = t

            # Startup-critical loads on sync, interleaved at d-strip-pair
            # granularity so chunk 0's d-outer matmuls can trickle behind the
            # DMA front.  fp8 halves first (chunk 0 runs its fp8 phase
            # first), then the bf16 halves.
            uatb = consts.tile([128, SD, HTOPB], bf16)
            uat8 = consts.tile([128, SD, H8], fp8)
            ktb0 = ktbp.tile([128, SD, schunk], bf16, tag="ktb", name="ktb_0_0")
            kt80 = kt8p.tile([128, SD, schunk], fp8, tag="kt8", name="kt8_0_0")
            ktb_tiles[0] = ktb0
            kt8_tiles[0] = kt80
            step = 2 if SD >= 2 else 1
            for g in range(0, SD, step):
                e = min(g + step, SD)
                nc.sync.dma_start(out=uat8[:, g:e, :], in_=uat8_in[:, g:e, :])
                nc.sync.dma_start(out=kt80[:, g:e, :], in_=kt8_in[0][:, g:e, :])
            for g in range(0, SD, step):
                e = min(g + step, SD)
                nc.sync.dma_start(out=uatb[:, g:e, :], in_=uatb_in[:, g:e, :])
                nc.sync.dma_start(out=ktb0[:, g:e, :], in_=ktb_in[0][:, g:e, :])
            if len(seq) > 1:
                load_ktg(1)
            # First kn chunks go on gpsimd BEHIND the critical path (they are
            # not needed until the deferred context of chunk 0/1/2), so they
            # don't delay the sync-queue ktg prefetches for positions 2-4.
            for p in range(min(3, len(seq))):
                load_kn(p, nc.gpsimd)

            # combined additive bias columns (Wa_b + Ua_b)
            comb = consts.tile([128, SM], fp32)
            nc.vector.tensor_tensor(
                out=comb, in0=wabc, in1=uabc, op=mybir.AluOpType.add
            )

            # wq staging + bias columns (filled mid-chunk-0, see emit_wq_bias)
            wq_sb = rows.tile([bpc, h], fp32, tag="wq")
            bias_cols = consts.tile([128, SM, bpc], fp32)

            def emit_wq_bias(pps0):
                # wq = q @ Wa^T and bias_cols[:, m, b] = wq[b].T + Wa_b + Ua_b.
                # Runs between chunk 0's fp8 and bf16 phases; all PSUM scratch
                # borrows regions of pps0 (= pps[0]) before m0's accumulation
                # overwrites the whole bank.
                for wh in range(NWH):
                    pw = pps0[:bpc, :WW]
                    for jj in range(SJ):
                        nc.tensor.matmul(
                            out=pw,
                            lhsT=qt[:, jj, :],
                            rhs=wat[:, jj, wh * WW : (wh + 1) * WW],
                            start=(jj == 0),
                            stop=(jj == SJ - 1),
                        )
                    nc.vector.tensor_copy(
                        out=wq_sb[:, wh * WW : (wh + 1) * WW], in_=pw
                    )
                for m in range(SM):
                    pt = pps0[:, m * bpc : (m + 1) * bpc]
                    nc.tensor.transpose(
                        out=pt,
                        in_=wq_sb[:bpc, m * 128 : (m + 1) * 128],
                        identity=ident[:bpc, :bpc],
                    )
                    nc.vector.tensor_scalar_add(
                        out=bias_cols[:, m, :], in0=pt, scalar1=comb[:, m : m + 1]
                    )

            # ---------------- main loop over (batch, chunk) ----------------
            state = {}

            def new_batch_state(b):
                state[b] = {
                    "exp_row": rows.tile(
                        [1, s], fp32, tag="exp_row", name=f"exp_row_{b}"
                    ),
                    "tparts": rows2.tile(
                        [1, NCH], fp32, tag="tparts", name=f"tparts_{b}"
                    ),
                    "ecols": rows2.tile(
                        [128, NST], bf16, tag="ecols", name=f"ecols_{b}"
                    ),
                    "cx": None,
                }

            def emit_transposes(pos):
                # transpose chunk c's exp slice into columns (the bf16 copy
                # lands while the current chunk's uk stream is still running)
                b, c = seq[pos]
                st = state[b]
                pscT = ps_sc.tile([128, SPC], fp32, tag="sc", name=f"pscT_{pos}")
                for g in range(SPC):
                    nc.tensor.transpose(
                        out=pscT[:, g : g + 1],
                        in_=st["exp_row"][
                            :1, c * schunk + g * 128 : c * schunk + (g + 1) * 128
                        ],
                        identity=ident[:1, :1],
                    )
                nc.vector.tensor_copy(
                    out=st["ecols"][:, c * SPC : (c + 1) * SPC], in_=pscT
                )

            def emit_finish(pos):
                # accumulate chunk c's context partials into ONE PSUM bank:
                # the NDC output chunks go to column groups 0/32/64/96 via
                # tile_position, so consecutive jd matmuls run concurrently
                # on disjoint 32-column strips of the PE array
                b, c = seq[pos]
                st = state[b]
                if c == 0:
                    st["cx"] = ps_cx.tile([128, DW], fp32, tag="cx", name=f"cx_{b}")
                for i in range(SPC):
                    for jd in range(NDC):
                        nc.tensor.matmul(
                            out=st["cx"][32 * jd : 32 * jd + 1, :],
                            lhsT=st["ecols"][:, c * SPC + i : c * SPC + i + 1],
                            rhs=kn_tiles[pos][:, i, jd * DW : (jd + 1) * DW],
                            start=(c == 0 and i == 0),
                            stop=(c == NCH - 1 and i == SPC - 1),
                            tile_position=(0, 32 * jd),
                            skip_group_check=True,
                        )
                if c == NCH - 1:
                    # scale finished rows out.  Mid-stream batches alternate
                    # DVE/ScalarE (overlaps with the next chunk's uk).  The
                    # final batch puts all four on DVE: its w_norm moved to
                    # ScalarE, and the cross-engine sem handoffs (~0.9us
                    # each) would otherwise serialize the tail anyway.
                    for jd in range(NDC):
                        half = "ctx_ev" if jd % 2 == 0 else "ctx_od"
                        idx = jd // 2
                        dst = st[half][:, idx * DW : (idx + 1) * DW]
                        if b == bpc - 1 or jd % 2 == 0:
                            nc.vector.tensor_scalar_mul(
                                out=dst,
                                in0=st["cx"][32 * jd : 32 * jd + 1, :],
                                scalar1=st["invt"],
                            )
                        else:
                            nc.scalar.activation(
                                out=dst,
                                in_=st["cx"][32 * jd : 32 * jd + 1, :],
                                func=AF.Copy,
                                scale=st["invt"],
                            )
                    q = nc.sync if b == bpc - 1 else nc.gpsimd
                    if NDC == 1:
                        q.dma_start(out=ctx_out[b : b + 1, :], in_=st["ctx_ev"])
                    else:
                        # strided DRAM APs: even tile -> chunks 0,2,..; odd -> 1,3,..
                        ev_ap = ctx_out[b : b + 1, :].rearrange(
                            "1 (n two w) -> 1 two n w", two=2, w=DW
                        )
                        q.dma_start(
                            out=ev_ap[:, 0],
                            in_=st["ctx_ev"].rearrange("1 (n w) -> 1 n w", w=DW),
                        )
                        q.dma_start(
                            out=ev_ap[:, 1],
                            in_=st["ctx_od"].rearrange("1 (n w) -> 1 n w", w=DW),
                        )

            def emit_uk_matmuls(pos, m, puk):
                # uk accumulation for h-tile m: tile 0 all-bf16, tile 1 mixed
                # (fp8 DoubleRow strips first, then nbf1 bf16 strips), tiles
                # 2+ all-fp8 DoubleRow
                if m == 0:
                    for dd in range(SD):
                        nc.tensor.matmul(
                            out=puk,
                            lhsT=uatb[:, dd, :128],
                            rhs=ktb_tiles[pos][:, dd, :],
                            start=(dd == 0),
                            stop=(dd == SD - 1),
                        )
                elif m == 1:
                    for dd in range(nbf1, SD, 2):
                        nc.tensor.matmul(
                            out=puk,
                            lhsT=uat8[:, dd : dd + 2, 0:128],
                            rhs=kt8_tiles[pos][:, dd : dd + 2, :],
                            start=(dd == nbf1),
                            stop=False,
                            perf_mode=DR,
                            skip_group_check=True,
                        )
                    for dd in range(nbf1):
                        nc.tensor.matmul(
                            out=puk,
                            lhsT=uatb[:, dd, 128:256],
                            rhs=ktb_tiles[pos][:, dd, :],
                            start=False,
                            stop=(dd == nbf1 - 1),
                            skip_group_check=True,
                        )
                else:
                    m8 = m - 1
                    for dd in range(0, SD, 2):
                        nc.tensor.matmul(
                            out=puk,
                            lhsT=uat8[:, dd : dd + 2, m8 * 128 : (m8 + 1) * 128],
                            rhs=kt8_tiles[pos][:, dd : dd + 2, :],
                            start=(dd == 0),
                            stop=(dd == SD - 2),
                            perf_mode=DR,
                        )

            for pos, (b, c) in enumerate(seq):
                if c == 0:
                    new_batch_state(b)
                # prefetch (kt pools bufs=2 -> one ahead; knp bufs=3 -> two ahead)
                if pos + 2 < len(seq) and (pos + 2) not in ktb_tiles:
                    load_ktg(pos + 2)
                if pos + 3 < len(seq) and (pos + 3) not in kn_tiles:
                    load_kn(pos + 3, nc.gpsimd)

                # fp8 m-tiles run first (their operands land first), then the
                # bf16-heavy tiles
                m_order = list(range(2, SM)) + [1, 0] if SM >= 2 else [0]
                ts_list = [None] * SM
                if pos == 0:
                    # chunk 0: d-outer with all m-psums live, fp8 phase then
                    # wq/bias (needs only gpsimd-loaded qt/wat) then bf16 phase
                    pmap = [
                        (ps_uk, "uk"), (ps_uk, "uk"), (ps_uk, "uk"),
                        (ps_sc, "sc"), (ps_sc, "sc"),
                        (ps_cx, "cx"), (ps_cx, "cx"), (ps_cx, "cx"),
                    ]
                    pps = []
                    for m in range(SM):
                        pool, tag = pmap[m * 8 // SM]
                        pps.append(
                            pool.tile(
                                [128, schunk], fp32, tag=tag, name=f"puk0_{m}"
                            )
                        )
                    # phase 1: all-fp8 work (m1's upper strips + tiles 2..7)
                    for dd in range(0, SD, 2):
                        for m in range(1, SM):
                            if m == 1:
                                if dd < nbf1:
                                    continue
                                lhs = uat8[:, dd : dd + 2, 0:128]
                                st_flag = dd == nbf1
                                sp_flag = False
                            else:
                                lhs = uat8[
                                    :, dd : dd + 2, (m - 1) * 128 : m * 128
                                ]
                                st_flag = dd == 0
                                sp_flag = dd == SD - 2
                            nc.tensor.matmul(
                                out=pps[m],
                                lhsT=lhs,
                                rhs=kt80[:, dd : dd + 2, :],
                                start=st_flag,
                                stop=sp_flag,
                                perf_mode=DR,
                                skip_group_check=True,
                            )
                    # wq + bias columns, borrowing pps[0]'s bank
                    emit_wq_bias(pps[0])
                    # phase 2: bf16 work (m1's first strips, then all of m0)
                    for dd in range(nbf1):
                        nc.tensor.matmul(
                            out=pps[1],
                            lhsT=uatb[:, dd, 128:256],
                            rhs=ktb0[:, dd, :],
                            start=False,
                            stop=(dd == nbf1 - 1),
                            skip_group_check=True,
                        )
                    for dd in range(SD):
                        nc.tensor.matmul(
                            out=pps[0],
                            lhsT=uatb[:, dd, :128],
                            rhs=ktb0[:, dd, :],
                            start=(dd == 0),
                            stop=(dd == SD - 1),
                            skip_group_check=True,
                        )
                    for m in m_order:
                        t_sb = tp.tile(
                            [128, schunk], bf16, tag="t", name=f"t_{pos}_{m}"
                        )
                        nc.scalar.activation(
                            out=t_sb,
                            in_=pps[m],
                            func=AF.Tanh,
                            bias=bias_cols[:, m, b : b + 1],
                            scale=1.0 if m == 0 else inv_s8,
                        )
                        ts_list[m] = t_sb
                else:
                    for mi, m in enumerate(m_order):
                        puk = ps_uk.tile([128, schunk], fp32, tag="uk")
                        emit_uk_matmuls(pos, m, puk)
                        t_sb = tp.tile(
                            [128, schunk], bf16, tag="t", name=f"t_{pos}_{m}"
                        )
                        nc.scalar.activation(
                            out=t_sb,
                            in_=puk,
                            func=AF.Tanh,
                            bias=bias_cols[:, m, b : b + 1],
                            scale=1.0 if m == 0 else inv_s8,
                        )
                        ts_list[m] = t_sb
                        if mi == 0 and pos > 0:
                            # hoist the previous chunk's score transposes
                            # here so their bf16 column cast (DVE) completes
                            # during this chunk's uk stream and the context
                            # matmuls below never wait on it
                            emit_transposes(pos - 1)

                def emit_scores(split):
                    # scores for this chunk.  split=True spreads the 8-strip
                    # contraction over 4 PE column groups (concurrent
                    # matmuls, partials at partitions 0/32/64/96) summed on
                    # DVE; the final chunk uses split=False so exp can read
                    # PSUM directly with no DVE chain on the tail
                    G = min(4, SM) if split else 1
                    gm = SM // G
                    psc = ps_sc.tile(
                        [128, schunk], fp32, tag="sc", name=f"psc_{pos}"
                    )
                    for r in range(gm):
                        for g in range(G):
                            m = g * gm + r
                            nc.tensor.matmul(
                                out=psc[32 * g : 32 * g + 1, :],
                                lhsT=vac[:, m : m + 1],
                                rhs=ts_list[m],
                                start=(r == 0),
                                stop=(r == gm - 1),
                                tile_position=(0, 32 * g),
                                skip_group_check=True,
                            )
                    if G > 1:
                        scs = rows2.tile(
                            [1, schunk], fp32, tag="scs", name=f"scs_{pos}"
                        )
                        nc.vector.tensor_copy(out=scs, in_=psc[0:1, :])
                        for g in range(1, G):
                            nc.vector.tensor_tensor(
                                out=scs,
                                in0=scs,
                                in1=psc[32 * g : 32 * g + 1, :],
                                op=mybir.AluOpType.add,
                            )
                    else:
                        scs = psc[0:1, :]
                    # exp row chunk (no max subtraction; scores are O(1))
                    # and the chunk's softmax partial sum
                    st = state[b]
                    nc.scalar.activation(
                        out=st["exp_row"][:, c * schunk : (c + 1) * schunk],
                        in_=scs,
                        func=AF.Exp,
                        accum_out=st["tparts"][:, c : c + 1],
                    )

                last = pos == len(seq) - 1
                if last:
                    emit_scores(split=False)
                if pos > 0:
                    emit_finish(pos - 1)
                if not last:
                    emit_scores(split=True)
                st = state[b]
                if last:
                    # hoist the final chunk's score transposes ahead of the
                    # softmax-denominator DVE ops so the ecols cast doesn't
                    # queue behind the 1.3us w_norm and stall the last
                    # context matmuls
                    emit_transposes(pos)
                if c == NCH - 1:
                    # softmax denominator + normalized weights can go out now;
                    # the raw exp_row stays untouched for the deferred context
                    tsum = rows2.tile([1, 1], fp32, tag="tsum", name=f"tsum_{b}")
                    nc.vector.reduce_sum(
                        out=tsum, in_=st["tparts"], axis=mybir.AxisListType.X
                    )
                    invt = rows2.tile([1, 1], fp32, tag="invt", name=f"invt_{b}")
                    nc.vector.reciprocal(out=invt, in_=tsum)
                    st["invt"] = invt
                    w_norm = rows.tile([1, s], fp32, tag="w_norm", name=f"wn_{b}")
                    if b == bpc - 1:
                        # final batch: ScalarE takes w_norm so DVE is free
                        # for the context normalize (see emit_finish)
                        nc.scalar.activation(
                            out=w_norm,
                            in_=st["exp_row"],
                            func=AF.Copy,
                            scale=invt,
                        )
                    else:
                        nc.vector.tensor_scalar_mul(
                            out=w_norm, in0=st["exp_row"], scalar1=invt
                        )
                    wq_ = nc.sync if b == bpc - 1 else nc.gpsimd
                    wq_.dma_start(out=w_out[b : b + 1, :], in_=w_norm)
                    st["ctx_ev"] = rows.tile(
                        [1, ((NDC + 1) // 2) * DW], fp32, tag="ctx_ev", name=f"ce_{b}"
                    )
                    if NDC > 1:
                        st["ctx_od"] = rows.tile(
                            [1, (NDC // 2) * DW], fp32, tag="ctx_od", name=f"co_{b}"
                        )

            emit_finish(len(seq) - 1)

    nc.compile()
    return nc


def _prep_core_inputs(q_last, keys_bf, keys_f8, b0, bpc, s, h, d2, schunk):
    """Host-side layout prep for one core: slice this core's batches and
    swizzle into the exact DRAM layouts the kernel DMAs from. Layout/dtype
    only -- no arithmetic."""
    import ml_dtypes

    bf16 = ml_dtypes.bfloat16
    f8 = ml_dtypes.float8_e4m3
    SD = d2 // 128
    SJ = h // 128
    NCH = s // schunk
    SPC = schunk // 128

    kn = np.empty((bpc * NCH, 128, SPC, d2), dtype=bf16)
    ktb = np.empty((bpc * NCH, 128, SD, schunk), dtype=bf16)
    kt8 = np.empty((bpc * NCH, 128, SD, schunk), dtype=f8)
    for b in range(bpc):
        ks = keys_bf[:, b0 + b, :]  # [s, d2] (strided view)
        k8 = keys_f8[:, b0 + b, :]
        # kn[b,c][p, i, x] = ks[c*schunk + i*128 + p, x]
        kn[b * NCH : (b + 1) * NCH] = ks.reshape(NCH, SPC, 128, d2).transpose(
            0, 2, 1, 3
        )
        # kt[b,c][p, dd, x] = ks[c*schunk + x, dd*128 + p]
        ktb[b * NCH : (b + 1) * NCH] = ks.reshape(NCH, schunk, SD, 128).transpose(
            0, 3, 2, 1
        )
        kt8[b * NCH : (b + 1) * NCH] = k8.reshape(NCH, schunk, SD, 128).transpose(
            0, 3, 2, 1
        )

    # qt[p, j, b] = q_last[b0+b, j*128+p]
    qt = np.ascontiguousarray(
        q_last[b0 : b0 + bpc].T.reshape(SJ, 128, bpc).transpose(1, 0, 2)
    ).astype(bf16)
    return {"qt": qt, "kn": kn, "ktb": ktb, "kt8": kt8}


def _make_in_maps(inputs):
    import ml_dtypes

    bf16 = ml_dtypes.bfloat16
    f8 = ml_dtypes.float8_e4m3
    q_last = np.ascontiguousarray(
        np.asarray(inputs["query"], dtype=np.float32)[:, -1, :]
    )  # [B, H]
    keys = np.asarray(inputs["keys"], dtype=np.float32)  # [S, B, 2H]
    keys_bf = keys.astype(bf16)
    keys_f8 = np.clip(keys, -240.0, 240.0).astype(f8)
    wa = np.asarray(inputs["Wa_w"], dtype=np.float32)  # [H, H]
    ua = np.asarray(inputs["Ua_w"], dtype=np.float32)  # [H, 2H]
    va = np.asarray(inputs["Va_w"], dtype=np.float32).reshape(1, H)
    wab = np.asarray(inputs["Wa_b"], dtype=np.float32).reshape(H)
    uab = np.asarray(inputs["Ua_b"], dtype=np.float32).reshape(H)

    # permute the h axis so |Va| is descending: the top h-tiles (most of
    # the Va^2 energy, i.e. of the output sensitivity) run in bf16, the rest
    # in fp8.  Pure layout change; scores/outputs are h-order invariant.
    perm = np.argsort(-np.abs(va[0]), kind="stable")
    wa = wa[perm]
    ua = ua[perm]
    va = va[:, perm]
    wab = wab[perm]
    uab = uab[perm]

    SD = D2 // 128
    SJ = H // 128
    SM = H // 128
    HTOPB = 256
    # uat[p, dd, j] = Ua_w[j, dd*128+p]; bf16 covers tiles 0-1, fp8 tiles 1+.
    # Tile 1 mixes fp8 (x FP8_SCALE) and bf16 strips in one PSUM, so its
    # bf16 columns carry the same power-of-two pre-scale (exact in bf16);
    # the tanh descale then applies uniformly.
    uat = np.ascontiguousarray(ua.T.reshape(SD, 128, H).transpose(1, 0, 2))
    uatb = np.concatenate(
        [uat[:, :, :128], uat[:, :, 128:HTOPB] * FP8_SCALE], axis=2
    ).astype(bf16)
    uat8 = np.clip(uat[:, :, 128:] * FP8_SCALE, -240.0, 240.0).astype(f8)
    # wat[p, jj, ho] = Wa_w[ho, jj*128+p]
    wat = np.ascontiguousarray(
        wa.T.reshape(SJ, 128, H).transpose(1, 0, 2)
    ).astype(bf16)
    # vac[p, m] = Va_w[0, m*128+p]
    vac = np.ascontiguousarray(va.reshape(SM, 128).T).astype(bf16)
    wabc = np.ascontiguousarray(wab.reshape(SM, 128).T)
    uabc = np.ascontiguousarray(uab.reshape(SM, 128).T)

    in_maps = []
    for c in range(NCORES):
        m = _prep_core_inputs(
            q_last, keys_bf, keys_f8, c * BPC, BPC, S, H, D2, 512
        )
        m.update(
            {
                "uatb": uatb,
                "uat8": uat8,
                "wat": wat,
                "vac": vac,
                "wabc": wabc,
                "uabc": uabc,
            }
        )
        in_maps.append(m)
    return in_maps


def run(inputs, trace=False, **kwargs):
    """Run on all 8 cores; returns ((context, weights), BassKernelResults)."""
    from concourse.bass_utils import run_bass_kernel_spmd

    if "nc" not in _CACHE:
        _CACHE["nc"] = _build()
    nc = _CACHE["nc"]
    in_maps = _make_in_maps(inputs)
    res = run_bass_kernel_spmd(
        nc, in_maps, core_ids=list(range(NCORES)), trace=trace, **kwargs
    )
    context = np.empty((B, 1, D2), dtype=np.float32)
    weights = np.empty((B, 1, S), dtype=np.float32)
    for c in range(NCORES):
        b0 = c * BPC
        context[b0 : b0 + BPC, 0, :] = res.results[c]["ctx"]
        weights[b0 : b0 + BPC, 0, :] = res.results[c]["wts"]
    return (context, weights), res


def kernel(**inputs):
    out, _ = run(inputs)
    return out
